# revision 22
# baseline (speedup 1.0000x reference)
"""Longformer layer stack (4 layers, sliding-window attention) on 8 TRN2 cores.

Sharding: data-parallel over batch (2) x sequence-parallel (4 blocks of 1024
tokens). Each core computes its sequence block; the banded attention needs a
W=256 token halo, exchanged between neighboring blocks with an AllGather after
each layer (layers 0-2), overlapped with the next layer's halo-independent
work (Q proj, middle K/V, attention chunks 1-2).

Residual stream x kept transposed ([dmodel, tokens]) in fp32; a bf16 shadow xb
(with halos) feeds all matmuls. All matmul operands are bf16 except the
LayerNorm sum-reductions (fp32r). Softmax normalization is deferred: raw
exp-sums accumulate into o, denominators batch into one reciprocal per layer.
"""
import sys

sys.path.insert(0, '/opt/trn_rl_repo')

import numpy as np
import ml_dtypes

import concourse.bass as bass
import concourse.mybir as mybir
import concourse.tile as tile
from concourse import bacc
from concourse import bass_utils

F32 = mybir.dt.float32
F32R = mybir.dt.float32r
BF16 = mybir.dt.bfloat16
I32 = mybir.dt.int32
AF = mybir.ActivationFunctionType
ALU = mybir.AluOpType

NH = 12          # heads
DH = 64          # head dim
HD = 768         # model dim
FF = 3072        # ffn dim
W = 256          # one-sided window
L = 4            # layers
B = 2
S = 4096
EPS = 1e-12
N_CORES = 8
T_OWN = 1024     # tokens per core
T_EXT = 1536     # with halos
FT = 6           # model-dim 128-tiles
FFT = 24         # ffn-dim 128-tiles
NCH = 4          # local chunks of 256 queries
P = 128


def _ln_transposed(nc, tc, tag, r_aps, ones_r, eps_sb, out_writes):
    """LayerNorm over the partition (feature) axis of transposed tiles.

    r_aps: FT fp32r SBUF APs [128, 512], normalized IN PLACE (mean/std only);
    out_writes(ft, src_ap) then emits the scale/bias output op(s)."""
    with tc.tile_pool(name=f"lnsb_{tag}", bufs=1) as sb:
        ss = sb.tile([1, 1024], F32, tag="ss", name=f"ss_{tag}")
        with tc.tile_pool(name=f"lnsq_{tag}", bufs=2) as sqp, \
             tc.tile_pool(name=f"lnps_{tag}", bufs=1, space="PSUM") as pp:
            sumx = pp.tile([1, 512], F32, tag="sumx", name=f"sumx_{tag}")
            sumsq = pp.tile([1, 512], F32, tag="sumsq", name=f"sumsq_{tag}")
            for ft in range(FT):
                sq = sqp.tile([P, 512], F32R, tag="sq", name=f"sq_{tag}")
                nc.scalar.activation(sq[:], r_aps[ft], AF.Square)
                nc.tensor.matmul(sumx[0:1, :], ones_r[:, 0:1], r_aps[ft],
                                 start=(ft == 0), stop=(ft == FT - 1))
                nc.tensor.matmul(sumsq[0:1, :], ones_r[:, 0:1], sq[:],
                                 start=(ft == 0), stop=(ft == FT - 1))
            nc.vector.tensor_scalar_add(ss[0:1, 0:512], sumx[:], 0.0)
            nc.vector.tensor_scalar_add(ss[0:1, 512:1024], sumsq[:], 0.0)
        stat = sb.tile([1, 1024], F32, tag="stat", name=f"stat_{tag}")
        nc.scalar.activation(stat[0:1, 0:512], ss[0:1, 0:512], AF.Identity,
                             scale=1.0 / HD)
        musq = sb.tile([1, 512], F32, tag="musq", name=f"musq_{tag}")
        nc.scalar.activation(musq[:], stat[0:1, 0:512], AF.Square)
        var = sb.tile([1, 512], F32, tag="var", name=f"var_{tag}")
        nc.vector.scalar_tensor_tensor(out=var[:], in0=ss[0:1, 512:1024],
                                       scalar=1.0 / HD, in1=musq[:],
                                       op0=ALU.mult, op1=ALU.subtract)
        sd = sb.tile([1, 512], F32, tag="sd", name=f"sd_{tag}")
        nc.scalar.activation(sd[:], var[:], AF.Sqrt, bias=eps_sb[0:1, :])
        nc.vector.reciprocal(stat[0:1, 512:1024], sd[:])
        stat_b = sb.tile([P, 1024], F32, tag="statb", name=f"statb_{tag}")
        nc.gpsimd.partition_broadcast(stat_b[:], stat[:], channels=P)
        for ft in range(FT):
            nc.vector.tensor_tensor(r_aps[ft], r_aps[ft], stat_b[:, 0:512],
                                    op=ALU.subtract)
            nc.vector.tensor_tensor(r_aps[ft], r_aps[ft], stat_b[:, 512:1024],
                                    op=ALU.mult)
            out_writes(ft, r_aps[ft])


def build_nc(n_layers=L):
    nc = bacc.Bacc("TRN2", target_bir_lowering=False, debug=False,
                   num_devices=N_CORES)
    dt_ = nc.dram_tensor
    t = {}
    t["emb"] = dt_("emb_word", [32000, HD], F32, kind="ExternalInput").ap()
    t["ids"] = dt_("ids", [P, 12], I32, kind="ExternalInput").ap()
    t["pos"] = dt_("pos", [T_EXT, HD], F32, kind="ExternalInput").ap()
    t["eln_s"] = dt_("eln_s", [HD], F32, kind="ExternalInput").ap()
    t["eln_b"] = dt_("eln_b", [HD], F32, kind="ExternalInput").ap()
    t["wq"] = dt_("wq", [L, FT, P, FT, P], BF16, kind="ExternalInput").ap()
    t["wk"] = dt_("wk", [L, FT, P, FT, P], BF16, kind="ExternalInput").ap()
    t["wv"] = dt_("wv", [L, HD, HD], BF16, kind="ExternalInput").ap()
    t["wo"] = dt_("wo", [L, FT, P, FT, P], BF16, kind="ExternalInput").ap()
    t["w1"] = dt_("w1", [L, FFT, P, FT, P], BF16, kind="ExternalInput").ap()
    t["w2"] = dt_("w2", [L, FF, HD], BF16, kind="ExternalInput").ap()
    for nm in ["bq", "bk", "bo", "b2", "ls1", "lb1", "ls2", "lb2"]:
        t[nm] = dt_(nm, [L, P, FT], F32, kind="ExternalInput").ap()
    t["b1"] = dt_("b1", [L, P, FFT], F32, kind="ExternalInput").ap()
    t["sel"] = dt_("sel", [NH, FT * P], F32, kind="ExternalInput").ap()
    t["ml"] = dt_("ml", [NCH, P, 512], BF16, kind="ExternalInput").ap()
    t["mr"] = dt_("mr", [NCH, P, 512], BF16, kind="ExternalInput").ap()
    t["halo_ids"] = dt_("halo_ids", [P, 12], I32, kind="ExternalInput").ap()
    t["out"] = dt_("out", [FT, P, T_OWN], F32, kind="ExternalOutput").ap()

    with tile.TileContext(nc) as tc:
        _build_body(nc, tc, n_layers, t)
    nc.compile()
    return nc


def _build_body(nc, tc, n_layers, t):
    from contextlib import ExitStack
    with ExitStack() as ctx:
        persist = ctx.enter_context(tc.tile_pool(name="persist", bufs=1))
        # residual stream x (own tokens, fp32) + bf16 shadow xb (with halos)
        x = [persist.tile([P, T_OWN], F32R, tag=f"x{ft}", name=f"x{ft}")
             for ft in range(FT)]
        xb = [persist.tile([P, T_EXT], BF16, tag=f"xb{ft}", name=f"xb{ft}")
              for ft in range(FT)]
        ml_sb = [persist.tile([P, 512], BF16, tag=f"ml{c}", name=f"ml{c}") for c in range(NCH)]
        mr_sb = [persist.tile([P, 512], BF16, tag=f"mr{c}", name=f"mr{c}") for c in range(NCH)]
        for c in range(NCH):
            nc.sync.dma_start(ml_sb[c][:], t["ml"][c])
            nc.sync.dma_start(mr_sb[c][:], t["mr"][c])
        ones_f = persist.tile([P, 1], F32, tag="ones_f", name="ones_f")
        nc.vector.memset(ones_f[:], 1.0)
        ones_r = persist.tile([P, 1], F32R, tag="ones_r", name="ones_r")
        nc.scalar.activation(ones_r[:], ones_f[:], AF.Identity)
        from concourse.masks import make_identity
        ident = persist.tile([P, P], F32, tag="ident", name="ident")
        make_identity(nc, ident[:])
        hid_sb = persist.tile([P, 12], I32, tag="hid", name="hid")
        nc.sync.dma_start(hid_sb[:], t["halo_ids"][:])
        sel_sb = persist.tile([NH, FT * P], F32, tag="sel", name="sel")
        nc.sync.dma_start(sel_sb[:], t["sel"][:])
        eps_sb = persist.tile([P, 1], F32, tag="eps", name="eps")
        nc.vector.memset(eps_sb[:], EPS)

        # ---- embedding + LN -> x (own fp32) / xb (ext bf16) ----
        with tc.tile_pool(name="emb_sb", bufs=1) as esb, \
             tc.tile_pool(name="emb_sb2", bufs=2) as esb2, \
             tc.tile_pool(name="emb_ps", bufs=2, space="PSUM") as eps_p:
            ids_sb = esb.tile([P, 12], I32, tag="ids", name="ids")
            nc.sync.dma_start(ids_sb[:], t["ids"][:])
            s_bc = esb.tile([P, HD], F32, tag="sbc", name="sbc")
            nc.sync.dma_start(s_bc[:], bass.AP(
                tensor=t["eln_s"].tensor, offset=0, ap=[[0, P], [1, HD]]))
            b_bc = esb.tile([P, HD], F32, tag="bbc", name="bbc")
            nc.sync.dma_start(b_bc[:], bass.AP(
                tensor=t["eln_b"].tensor, offset=0, ap=[[0, P], [1, HD]]))
            e = [esb.tile([P, HD], F32, tag=f"e{tt}", name=f"e{tt}") for tt in range(12)]
            for tt in range(12):
                nc.gpsimd.indirect_dma_start(
                    out=e[tt][:], out_offset=None, in_=t["emb"][:],
                    in_offset=bass.IndirectOffsetOnAxis(
                        ap=ids_sb[:, tt:tt + 1], axis=0))
                p_sb = esb2.tile([P, HD], F32, tag="pos", name="pos")
                nc.sync.dma_start(p_sb[:], t["pos"][tt * P:(tt + 1) * P, :])
                nc.vector.tensor_tensor(e[tt][:], e[tt][:], p_sb[:], op=ALU.add)
                stats = esb2.tile([P, 3, nc.vector.BN_STATS_DIM], F32,
                                  tag="bst", name="bst")
                er = e[tt][:].rearrange("p (g d) -> p g d", g=3)
                for g in range(3):
                    nc.vector.bn_stats(stats[:, g, :], er[:, g, :])
                mv = esb2.tile([P, nc.vector.BN_AGGR_DIM], F32, tag="bag", name="bag")
                nc.vector.bn_aggr(mv[:], stats[:])
                sd = esb2.tile([P, 1], F32, tag="bsd", name="bsd")
                nc.scalar.activation(sd[:], mv[:, 1:2], AF.Sqrt, bias=eps_sb[:])
                rstd = esb2.tile([P, 1], F32, tag="brstd", name="brstd")
                nc.vector.reciprocal(rstd[:], sd[:])
                nc.vector.tensor_scalar(out=e[tt][:], in0=e[tt][:],
                                        scalar1=mv[:, 0:1], scalar2=rstd[:],
                                        op0=ALU.subtract, op1=ALU.mult)
                nc.vector.tensor_tensor(e[tt][:], e[tt][:], s_bc[:], op=ALU.mult)
                nc.vector.tensor_tensor(e[tt][:], e[tt][:], b_bc[:], op=ALU.add)
            for ft in range(FT):
                tr = eps_p.tile([P, T_EXT], F32, tag="tr", name="tr")
                for tt in range(12):
                    nc.tensor.transpose(tr[:, tt * P:(tt + 1) * P],
                                        e[tt][:, ft * P:(ft + 1) * P], ident[:])
                nc.scalar.activation(xb[ft][:], tr[:], AF.Identity)
                nc.scalar.activation(x[ft][:], tr[:, W:W + T_OWN], AF.Identity)

        for l in range(n_layers):
            _layer(nc, tc, t, l, x, xb, ml_sb, mr_sb, ones_r, eps_sb, hid_sb,
                   sel_sb, first=(l == 0), exchange=(l < n_layers - 1))

        for ft in range(FT):
            nc.gpsimd.dma_start(t["out"][ft], x[ft][:])


def _attn_chunk(nc, c, kT, qT, v, ml_sb, mr_sb, o, den2, bsb, bps, bps2):
    for h in range(NH):
        ft, po = h // 2, (h % 2) * 64
        sps = bps.tile([P, 6 * W], F32, tag="sps", name="sps")
        for w in range(6):
            nc.tensor.matmul(
                sps[:, w * W:(w + 1) * W],
                kT[ft][po:po + 64, (c * 2 + w) * P:(c * 2 + w + 1) * P],
                qT[ft][po:po + 64, c * W:(c + 1) * W],
                start=True, stop=True)
        ex = bsb.tile([P, 6 * W], BF16, tag="ex", name="ex")
        nc.scalar.activation(ex[:], sps[:], AF.Exp)
        nc.vector.tensor_tensor(ex[:, 0:512], ex[:, 0:512],
                                ml_sb[c][:], op=ALU.mult)
        nc.vector.tensor_tensor(ex[:, 1024:1536], ex[:, 1024:1536],
                                mr_sb[c][:], op=ALU.mult)
        ops = bps2.tile([P, W], F32, tag="ops", name="ops")
        for w in range(6):
            nc.tensor.matmul(
                ops[0:65, :], v[c * 2 + w][:, h, :], ex[:, w * W:(w + 1) * W],
                start=(w == 0), stop=(w == 5))
        nc.vector.tensor_scalar_add(o[ft][po:po + 64, c * W:(c + 1) * W],
                                    ops[0:64, :], 0.0)
        # denominator row -> staging (partition 0) -> DMA into den2[h]
        st = bsb.tile([1, W], F32, tag="st", name="st")
        nc.vector.tensor_scalar_add(st[:], ops[64:65, :], 0.0)
        nc.sync.dma_start(den2[h:h + 1, c * W:(c + 1) * W], st[:])


def _layer(nc, tc, t, l, x, xb, ml_sb, mr_sb, ones_r, eps_sb, hid_sb,
           sel_sb, first, exchange):
    from contextlib import ExitStack
    with ExitStack() as ctx:
        lsb = ctx.enter_context(tc.tile_pool(name=f"lsb{l}", bufs=1))
        edram = None
        if exchange:
            edram = ctx.enter_context(
                tc.tile_pool(name=f"pe_dram{l}", bufs=1, space="DRAM"))

        def bias_tile(name, n=FT):
            bt = lsb.tile([P, n], F32, tag=f"b_{name}", name=f"b_{name}")
            nc.sync.dma_start(bt[:], t[name][l])
            return bt
        bq_sb = bias_tile("bq"); bk_sb = bias_tile("bk"); bo_sb = bias_tile("bo")
        b1_sb = bias_tile("b1", FFT); b2_sb = bias_tile("b2")
        ls1_sb = bias_tile("ls1"); lb1_sb = bias_tile("lb1")
        ls2_sb = bias_tile("ls2"); lb2_sb = bias_tile("lb2")

        qT = [lsb.tile([P, T_OWN], BF16, tag=f"qT{i}", name=f"qT{i}") for i in range(FT)]
        kT = [lsb.tile([P, T_EXT], BF16, tag=f"kT{i}", name=f"kT{i}") for i in range(FT)]
        o = [lsb.tile([P, T_OWN], BF16, tag=f"o{i}", name=f"o{i}") for i in range(FT)]
        y = [lsb.tile([P, T_OWN], F32R, tag=f"y{i}", name=f"y{i}") for i in range(FT)]
        r2 = [lsb.tile([P, 512], F32R, tag=f"r2_{i}", name=f"r2_{i}") for i in range(FT)]
        den2 = lsb.tile([NH, T_OWN], F32, tag="den2", name="den2")
        rec = lsb.tile([NH, T_OWN], F32, tag="rec", name="rec")
        r1 = y  # post-attention residual, normalized in place by LN1 into y
        yb = qT  # qT is dead after attention; reuse as the bf16 LN1 output

        # per-layer weights kept resident: wk (reused for halo cols), wv
        wk_all = [lsb.tile([P, HD], BF16, tag=f"wk{i}", name=f"wk{i}")
                  for i in range(FT)]
        wv_all = [lsb.tile([P, HD], BF16, tag=f"wv{i}", name=f"wv{i}")
                  for i in range(FT)]
        for i in range(FT):
            nc.sync.dma_start(wk_all[i][:], t["wk"][l, i])
            nc.sync.dma_start(wv_all[i][:], t["wv"][l, i * P:(i + 1) * P, :])

        with ExitStack() as vctx:
            vpool = vctx.enter_context(tc.tile_pool(name=f"vp{l}", bufs=1))
            v = [vpool.tile([P, NH, 65], BF16, tag=f"v{i}", name=f"v{i}")
                 for i in range(12)]

            # ---- phase A1: Q projection (own tokens, halo-independent) ----
            with tc.tile_pool(name=f"pa_sb{l}", bufs=4) as asb, \
                 tc.tile_pool(name=f"pa_ps{l}", bufs=4, space="PSUM") as aps, \
                 tc.tile_pool(name=f"pa_psv{l}", bufs=2, space="PSUM") as vps:
                for mt in range(FT):
                    wq_sb = asb.tile([P, HD], BF16, tag="wqs", name="wqs")
                    nc.sync.dma_start(wq_sb[:], t["wq"][l, mt])
                    for h2 in range(2):
                        ps = aps.tile([P, 512], F32, tag="pp", name="pp")
                        for kt in range(FT):
                            nc.tensor.matmul(
                                ps[:], wq_sb[:, kt * P:(kt + 1) * P],
                                xb[kt][:, W + h2 * 512:W + (h2 + 1) * 512],
                                start=(kt == 0), stop=(kt == FT - 1))
                        nc.scalar.activation(qT[mt][:, h2 * 512:(h2 + 1) * 512],
                                             ps[:], AF.Identity,
                                             bias=bq_sb[:, mt:mt + 1])
                # ---- A2: K projection, middle (own) columns ----
                for mt in range(FT):
                    for seg in range(2):
                        ps = aps.tile([P, 512], F32, tag="pp", name="pp")
                        for kt in range(FT):
                            nc.tensor.matmul(
                                ps[:], wk_all[mt][:, kt * P:(kt + 1) * P],
                                xb[kt][:, W + seg * 512:W + (seg + 1) * 512],
                                start=(kt == 0), stop=(kt == FT - 1))
                        nc.scalar.activation(
                            kT[mt][:, W + seg * 512:W + (seg + 1) * 512],
                            ps[:], AF.Identity, bias=bk_sb[:, mt:mt + 1])
                # ---- A3: V projection, own token tiles ----
                for tt in range(2, 10):
                    for hf in range(2):
                        ps = vps.tile([P, 384], F32, tag="ppv", name="ppv")
                        for kt in range(FT):
                            nc.tensor.matmul(
                                ps[:], xb[kt][:, tt * P:(tt + 1) * P],
                                wv_all[kt][:, hf * 384:(hf + 1) * 384],
                                start=(kt == 0), stop=(kt == FT - 1))
                        nc.scalar.activation(
                            v[tt][:, hf * 6:(hf + 1) * 6, 0:64],
                            ps[:].rearrange("p (h d) -> p h d", h=6), AF.Identity)
                    nc.vector.memset(v[tt][:, :, 64:65], 1.0)

            # ---- B-mid: attention chunks 1,2 (no halo dependency) ----
            with tc.tile_pool(name=f"pb_sb{l}", bufs=3) as bsb, \
                 tc.tile_pool(name=f"pb_ps{l}", bufs=2, space="PSUM") as bps, \
                 tc.tile_pool(name=f"pb_ps2{l}", bufs=2, space="PSUM") as bps2:
                for c in (1, 2):
                    _attn_chunk(nc, c, kT, qT, v, ml_sb, mr_sb, o, den2,
                                bsb, bps, bps2)

            # ---- A4/A5: K/V projections for halo columns ----
            with tc.tile_pool(name=f"ph_ps{l}", bufs=4, space="PSUM") as hps, \
                 tc.tile_pool(name=f"ph_psv{l}", bufs=2, space="PSUM") as hvs:
                for mt in range(FT):
                    for side in range(2):
                        cs = slice(0, W) if side == 0 else slice(W + T_OWN, T_EXT)
                        ps = hps.tile([P, W], F32, tag="ph", name="ph")
                        for kt in range(FT):
                            nc.tensor.matmul(
                                ps[:], wk_all[mt][:, kt * P:(kt + 1) * P],
                                xb[kt][:, cs],
                                start=(kt == 0), stop=(kt == FT - 1))
                        nc.scalar.activation(kT[mt][:, cs], ps[:], AF.Identity,
                                             bias=bk_sb[:, mt:mt + 1])
                for tt in (0, 1, 10, 11):
                    for hf in range(2):
                        ps = hvs.tile([P, 384], F32, tag="phv", name="phv")
                        for kt in range(FT):
                            nc.tensor.matmul(
                                ps[:], xb[kt][:, tt * P:(tt + 1) * P],
                                wv_all[kt][:, hf * 384:(hf + 1) * 384],
                                start=(kt == 0), stop=(kt == FT - 1))
                        nc.scalar.activation(
                            v[tt][:, hf * 6:(hf + 1) * 6, 0:64],
                            ps[:].rearrange("p (h d) -> p h d", h=6), AF.Identity)
                    nc.vector.memset(v[tt][:, :, 64:65], 1.0)

            # ---- B-edge: attention chunks 0,3 ----
            with tc.tile_pool(name=f"pbe_sb{l}", bufs=3) as bsb, \
                 tc.tile_pool(name=f"pbe_ps{l}", bufs=2, space="PSUM") as bps, \
                 tc.tile_pool(name=f"pbe_ps2{l}", bufs=2, space="PSUM") as bps2:
                for c in (0, 3):
                    _attn_chunk(nc, c, kT, qT, v, ml_sb, mr_sb, o, den2,
                                bsb, bps, bps2)

        # ---- softmax normalization (batched) ----
        nc.vector.reciprocal(rec[:], den2[:])
        with tc.tile_pool(name=f"pr{l}", bufs=2, space="PSUM") as rp:
            for ft in range(FT):
                # recb[m, q] = rec[2ft + (m >= 64), q] via selector matmul
                recb = rp.tile([P, T_OWN], F32, tag="recb", name="recb")
                for j in range(2):
                    nc.tensor.matmul(recb[:, j * 512:(j + 1) * 512],
                                     sel_sb[:, ft * P:(ft + 1) * P],
                                     rec[:, j * 512:(j + 1) * 512],
                                     start=True, stop=True)
                nc.vector.tensor_tensor(o[ft][:], o[ft][:], recb[:],
                                        op=ALU.mult)

        # ---- phase C: O-proj + residual (r1 <- x + O@Wo + bo) ----
        with tc.tile_pool(name=f"pc_sb{l}", bufs=3) as csb, \
             tc.tile_pool(name=f"pc_ps{l}", bufs=4, space="PSUM") as cps:
            for mt in range(FT):
                wo_sb = csb.tile([P, HD], BF16, tag="wos", name="wos")
                nc.sync.dma_start(wo_sb[:], t["wo"][l, mt])
                for h2 in range(2):
                    ps = cps.tile([P, 512], F32, tag="ppo", name="ppo")
                    for kt in range(FT):
                        nc.tensor.matmul(
                            ps[:], wo_sb[:, kt * P:(kt + 1) * P],
                            o[kt][:, h2 * 512:(h2 + 1) * 512],
                            start=(kt == 0), stop=(kt == FT - 1))
                    nc.vector.scalar_tensor_tensor(
                        out=r1[mt][:, h2 * 512:(h2 + 1) * 512], in0=ps[:],
                        scalar=bo_sb[:, mt:mt + 1],
                        in1=x[mt][:, h2 * 512:(h2 + 1) * 512],
                        op0=ALU.add, op1=ALU.add)

        # ---- LN1 (per token-half): y <- LN(r1)*s+b (in place), yb bf16 ----
        for h2 in range(2):
            cs = slice(h2 * 512, (h2 + 1) * 512)

            def ln1_out(ft, src, cs=cs):
                nc.scalar.activation(yb[ft][:, cs], src, AF.Identity,
                                     scale=ls1_sb[:, ft:ft + 1],
                                     bias=lb1_sb[:, ft:ft + 1])
                nc.scalar.activation(src, src, AF.Identity,
                                     scale=ls1_sb[:, ft:ft + 1],
                                     bias=lb1_sb[:, ft:ft + 1])
            _ln_transposed(nc, tc, f"l1_{l}_{h2}",
                           [r1[ft][:, cs] for ft in range(FT)],
                           ones_r, eps_sb, ln1_out)

        # ---- phase D: FFN + residual + LN2 -> x/xb ----
        for h2 in range(2):
            cs = slice(h2 * 512, (h2 + 1) * 512)
            with ExitStack() as dctx:
                dsb = dctx.enter_context(
                    tc.tile_pool(name=f"pd_sb{l}_{h2}", bufs=4))
                zps = dctx.enter_context(
                    tc.tile_pool(name=f"pd_psz{l}_{h2}", bufs=1, space="PSUM"))
                fps = dctx.enter_context(
                    tc.tile_pool(name=f"pd_psf{l}_{h2}", bufs=2, space="PSUM"))
                zp = [zps.tile([P, 512], F32, tag=f"z{mt}", name=f"z{mt}") for mt in range(FT)]
                for ms in range(FFT):
                    w1_sb = dsb.tile([P, HD], BF16, tag="w1s", name="w1s")
                    nc.sync.dma_start(w1_sb[:], t["w1"][l, ms])
                    fp = fps.tile([P, 512], F32, tag="fp", name="fp")
                    for kt in range(FT):
                        nc.tensor.matmul(fp[:], w1_sb[:, kt * P:(kt + 1) * P],
                                         yb[kt][:, cs], start=(kt == 0),
                                         stop=(kt == FT - 1))
                    f_sb = dsb.tile([P, 512], BF16, tag="fsb", name="fsb")
                    nc.scalar.activation(f_sb[:], fp[:], AF.Gelu,
                                         bias=b1_sb[:, ms:ms + 1])
                    w2_sb = dsb.tile([P, HD], BF16, tag="w2s", name="w2s")
                    nc.sync.dma_start(w2_sb[:],
                                      t["w2"][l, ms * P:(ms + 1) * P, :])
                    for mt in range(FT):
                        nc.tensor.matmul(zp[mt][:],
                                         w2_sb[:, mt * P:(mt + 1) * P],
                                         f_sb[:], start=(ms == 0),
                                         stop=(ms == FFT - 1))
                for mt in range(FT):
                    nc.vector.scalar_tensor_tensor(
                        out=r2[mt][:], in0=zp[mt][:],
                        scalar=b2_sb[:, mt:mt + 1], in1=y[mt][:, cs],
                        op0=ALU.add, op1=ALU.add)

            def ln2_out(ft, src, h2=h2):
                off = h2 * 512
                nc.scalar.activation(xb[ft][:, W + off:W + off + 512], src,
                                     AF.Identity, scale=ls2_sb[:, ft:ft + 1],
                                     bias=lb2_sb[:, ft:ft + 1])
                nc.scalar.activation(x[ft][:, off:off + 512], src,
                                     AF.Identity, scale=ls2_sb[:, ft:ft + 1],
                                     bias=lb2_sb[:, ft:ft + 1])
            _ln_transposed(nc, tc, f"l2_{l}_{h2}",
                           [r2[ft][:] for ft in range(FT)],
                           ones_r, eps_sb, ln2_out)

            # stage the ready boundary for the halo exchange ASAP
            if exchange and h2 == 0:
                b_in = edram.tile([2, FT, P, W], BF16, tag="bin", name="bin")
                for ft in range(FT):
                    nc.sync.dma_start(b_in[0, ft], xb[ft][:, W:2 * W])

        # ---- phase E: halo exchange (overlapped with next layer) ----
        if exchange:
            b_out = edram.tile([4 * 2 * FT * P, W], BF16, tag="bout",
                               name="bout")
            for ft in range(FT):
                nc.sync.dma_start(b_in[1, ft], xb[ft][:, T_OWN:T_OWN + W])
            nc.gpsimd.collective_compute(
                "AllGather", ALU.bypass,
                replica_groups=[[0, 1, 2, 3], [4, 5, 6, 7]],
                ins=[b_in[:].opt()], outs=[b_out[:].opt()])
            for side in range(2):
                for ft in range(FT):
                    dst = (xb[ft][:, 0:W] if side == 0
                           else xb[ft][:, T_OWN + W:T_EXT])
                    nc.gpsimd.indirect_dma_start(
                        out=dst, out_offset=None, in_=b_out[:],
                        in_offset=bass.IndirectOffsetOnAxis(
                            ap=hid_sb[:, side * FT + ft:side * FT + ft + 1],
                            axis=0))


# ---------------- host side ----------------

def _blocked(w, n_k, n_m):
    """[n_k*128, n_m*128] -> [n_m, 128, n_k, 128] (lhsT strips by out-tile)."""
    return np.ascontiguousarray(
        w.reshape(n_k, P, n_m, P).transpose(2, 1, 0, 3))


def _bias_lay(b, n):
    return np.ascontiguousarray(b.reshape(n, P).T)


def prepare(inputs):
    """Build per-core in_maps from full inputs."""
    ids_full = np.asarray(inputs["input_ids"]).astype(np.int32)
    am = np.asarray(inputs["attention_mask"]).astype(np.int32)
    emb_word = np.asarray(inputs["emb_word"], dtype=np.float32)
    emb_pos = np.asarray(inputs["emb_pos"], dtype=np.float32)
    Wq = np.asarray(inputs["Wq"], np.float32) / np.sqrt(DH)
    bq = np.asarray(inputs["bq"], np.float32) / np.sqrt(DH)
    Wk = np.asarray(inputs["Wk"], np.float32)
    bk = np.asarray(inputs["bk"], np.float32)
    Wv = np.asarray(inputs["Wv"], np.float32)
    bv = np.asarray(inputs["bv"], np.float32)
    Wo = np.asarray(inputs["Wo"], np.float32)
    bo = np.asarray(inputs["bo"], np.float32)
    W1 = np.asarray(inputs["W1"], np.float32)
    b1 = np.asarray(inputs["b1"], np.float32)
    W2 = np.asarray(inputs["W2"], np.float32)
    b2 = np.asarray(inputs["b2"], np.float32)
    assert np.all(am == 1), "general attention_mask needs mid-tile masks too"
    bf = ml_dtypes.bfloat16

    shared = {
        "emb_word": emb_word,
        "eln_s": np.asarray(inputs["emb_ln_s"], np.float32),
        "eln_b": np.asarray(inputs["emb_ln_b"], np.float32),
        "wq": np.stack([_blocked(Wq[i], FT, FT) for i in range(L)]).astype(bf),
        "wk": np.stack([_blocked(Wk[i], FT, FT) for i in range(L)]).astype(bf),
        "wv": Wv.astype(bf),
        "wo": np.stack([_blocked(Wo[i], FT, FT) for i in range(L)]).astype(bf),
        "w1": np.stack([_blocked(W1[i], FT, FFT) for i in range(L)]).astype(bf),
        "w2": W2.astype(bf),
        "bq": np.stack([_bias_lay(bq[i], FT) for i in range(L)]),
        "bk": np.stack([_bias_lay(bk[i], FT) for i in range(L)]),
        "bo": np.stack([_bias_lay(bv[i] @ Wo[i] + bo[i], FT)
                        for i in range(L)]),
        "b1": np.stack([_bias_lay(b1[i], FFT) for i in range(L)]),
        "b2": np.stack([_bias_lay(b2[i], FT) for i in range(L)]),
        "ls1": np.stack([_bias_lay(np.asarray(inputs["ln1_s"], np.float32)[i],
                                   FT) for i in range(L)]),
        "lb1": np.stack([_bias_lay(np.asarray(inputs["ln1_b"], np.float32)[i],
                                   FT) for i in range(L)]),
        "ls2": np.stack([_bias_lay(np.asarray(inputs["ln2_s"], np.float32)[i],
                                   FT) for i in range(L)]),
        "lb2": np.stack([_bias_lay(np.asarray(inputs["ln2_b"], np.float32)[i],
                                   FT) for i in range(L)]),
    }
    sel = np.zeros((NH, FT * P), np.float32)
    for ft in range(FT):
        sel[2 * ft, ft * P:ft * P + 64] = 1.0
        sel[2 * ft + 1, ft * P + 64:(ft + 1) * P] = 1.0
    shared["sel"] = sel

    in_maps = []
    i_idx = np.arange(W)
    for core in range(N_CORES):
        b, sb = core // 4, core % 4
        s0 = sb * T_OWN
        ext_pos = np.clip(np.arange(s0 - W, s0 + T_OWN + W), 0, S - 1)
        m = dict(shared)
        m["ids"] = np.ascontiguousarray(
            ids_full[b, ext_pos].reshape(12, P).T)
        m["pos"] = np.ascontiguousarray(emb_pos[ext_pos])
        # masks: global chunk gc, window key j in [0,768), query i in [0,256):
        #   key_abs = gc*W - W + j ; allowed = |j - W - i| <= W
        #             & 0 <= key_abs < S & attention_mask[b, key_abs]
        mlm = np.zeros((NCH, P, 512), np.float32)
        mrm = np.zeros((NCH, P, 512), np.float32)
        for c in range(NCH):
            gc = sb * NCH + c
            for kt2 in range(2):
                for mm_, j0 in ((mlm, 0), (mrm, 512)):
                    j = j0 + kt2 * P + np.arange(P)[:, None]
                    key_abs = gc * W - W + j
                    ok = (np.abs(j - W - i_idx[None, :]) <= W)
                    ok &= (key_abs >= 0) & (key_abs < S)
                    ok &= am[b, np.clip(key_abs, 0, S - 1)] > 0
                    mm_[c, :, kt2 * W:(kt2 + 1) * W] = ok
        m["ml"] = mlm.astype(bf)
        m["mr"] = mrm.astype(bf)
        # halo row ids into the gathered [4, 2, FT, 128, W] row table
        hid = np.zeros((2, FT, P), np.int64)
        for side in range(2):
            nb = sb - 1 if side == 0 else sb + 1
            if 0 <= nb <= 3:
                osd = 1 - side  # left halo <- neighbor's right block
                for ft in range(FT):
                    hid[side, ft] = ((nb * 2 + osd) * FT + ft) * P \
                        + np.arange(P)
            else:
                for ft in range(FT):
                    hid[side, ft] = ((sb * 2 + side) * FT + ft) * P \
                        + np.arange(P)
        m["halo_ids"] = np.ascontiguousarray(
            hid.reshape(12, P).T.astype(np.int32))
        in_maps.append(m)
    return in_maps


_NC_CACHE = {}


def get_nc(n_layers=L):
    if n_layers not in _NC_CACHE:
        _NC_CACHE[n_layers] = build_nc(n_layers)
    return _NC_CACHE[n_layers]


def run(inputs, n_layers=L, trace=False):
    nc = get_nc(n_layers)
    in_maps = prepare(inputs)
    res = bass_utils.run_bass_kernel_spmd(
        nc, in_maps, core_ids=list(range(N_CORES)), trace=trace)
    outs = np.empty((B, S, HD), np.float32)
    for core in range(N_CORES):
        b, sb = core // 4, core % 4
        ot = res.results[core]["out"]  # [FT, 128, T_OWN]
        outs[b, sb * T_OWN:(sb + 1) * T_OWN] = ot.reshape(HD, T_OWN).T
    return outs, res


def kernel(**inputs) -> np.ndarray:
    out, _ = run(inputs)
    return out


# revision 40
# speedup vs baseline: 1.0351x; 1.0351x over previous
"""Longformer layer stack (4 layers, sliding-window attention) on 8 TRN2 cores.

Sharding: data-parallel over batch (2) x sequence-parallel (4 blocks of 1024
tokens). Each core computes its sequence block; the banded attention needs a
W=256 token halo, exchanged between neighboring blocks with an AllGather after
each layer (layers 0-2), overlapped with the next layer's halo-independent
work (Q proj, middle K/V, attention chunks 1-2).

Residual stream x kept transposed ([dmodel, tokens]) in fp32; a bf16 shadow xb
(with halos) feeds all matmuls. All matmul operands are bf16 except the
LayerNorm sum-reductions (fp32r). Softmax normalization is deferred: raw
exp-sums accumulate into o, denominators batch into one reciprocal per layer.
"""
import sys

sys.path.insert(0, '/opt/trn_rl_repo')

import numpy as np
import ml_dtypes

import concourse.bass as bass
import concourse.mybir as mybir
import concourse.tile as tile
from concourse import bacc
from concourse import bass_utils

F32 = mybir.dt.float32
F32R = mybir.dt.float32r
BF16 = mybir.dt.bfloat16
I32 = mybir.dt.int32
AF = mybir.ActivationFunctionType
ALU = mybir.AluOpType

NH = 12          # heads
DH = 64          # head dim
HD = 768         # model dim
FF = 3072        # ffn dim
W = 256          # one-sided window
L = 4            # layers
B = 2
S = 4096
EPS = 1e-12
N_CORES = 8
T_OWN = 1024     # tokens per core
T_EXT = 1536     # with halos
FT = 6           # model-dim 128-tiles
FFT = 24         # ffn-dim 128-tiles
NCH = 4          # local chunks of 256 queries
P = 128


def _ln_transposed(nc, tc, tag, r_aps, ones_r, eps_sb, out_writes, ncols=512):
    """LayerNorm over the partition (feature) axis of transposed tiles.

    r_aps: FT fp32r SBUF APs [128, ncols], normalized IN PLACE (mean/std
    only); out_writes(ft, src_ap) then emits the scale/bias output op(s)."""
    nseg = ncols // 512
    with tc.tile_pool(name=f"lnsb_{tag}", bufs=1) as sb:
        ss = sb.tile([1, 2 * ncols], F32, tag="ss", name=f"ss_{tag}")
        with tc.tile_pool(name=f"lnsq_{tag}", bufs=2 if nseg == 1 else 1) as sqp, \
             tc.tile_pool(name=f"lnps_{tag}", bufs=1, space="PSUM") as pp:
            sumx = pp.tile([1, ncols], F32, tag="sumx", name=f"sumx_{tag}")
            sumsq = pp.tile([1, ncols], F32, tag="sumsq", name=f"sumsq_{tag}")
            for ft in range(FT):
                sq = sqp.tile([P, ncols], F32R, tag="sq", name=f"sq_{tag}")
                nc.scalar.activation(sq[:], r_aps[ft], AF.Square)
                for j in range(nseg):
                    cs = slice(j * 512, (j + 1) * 512)
                    nc.tensor.matmul(sumx[0:1, cs], ones_r[:, 0:1],
                                     r_aps[ft][:, cs],
                                     start=(ft == 0), stop=(ft == FT - 1))
                    nc.tensor.matmul(sumsq[0:1, cs], ones_r[:, 0:1],
                                     sq[:, cs],
                                     start=(ft == 0), stop=(ft == FT - 1))
            nc.vector.tensor_scalar_add(ss[0:1, 0:ncols], sumx[:], 0.0)
            nc.vector.tensor_scalar_add(ss[0:1, ncols:2 * ncols], sumsq[:], 0.0)
        # stat = [mu || sd]; normalize is (r - mu) / sd
        stat = sb.tile([1, 2 * ncols], F32, tag="stat", name=f"stat_{tag}")
        nc.scalar.activation(stat[0:1, 0:ncols], ss[0:1, 0:ncols], AF.Identity,
                             scale=1.0 / HD)
        musq = sb.tile([1, ncols], F32, tag="musq", name=f"musq_{tag}")
        nc.scalar.activation(musq[:], stat[0:1, 0:ncols], AF.Square)
        var = sb.tile([1, ncols], F32, tag="var", name=f"var_{tag}")
        nc.vector.scalar_tensor_tensor(out=var[:], in0=ss[0:1, ncols:2 * ncols],
                                       scalar=1.0 / HD, in1=musq[:],
                                       op0=ALU.mult, op1=ALU.subtract)
        sd = sb.tile([1, ncols], F32, tag="sd", name=f"sd_{tag}")
        nc.scalar.activation(sd[:], var[:], AF.Sqrt, bias=eps_sb[0:1, :])
        nc.vector.reciprocal(stat[0:1, ncols:2 * ncols], sd[:])
        stat_b = sb.tile([P, 2 * ncols], F32, tag="statb", name=f"statb_{tag}")
        nc.gpsimd.partition_broadcast(stat_b[:], stat[:], channels=P)
        for ft in range(FT):
            nc.vector.tensor_tensor(r_aps[ft], r_aps[ft], stat_b[:, 0:ncols],
                                    op=ALU.subtract)
            nc.vector.tensor_tensor(r_aps[ft], r_aps[ft],
                                    stat_b[:, ncols:2 * ncols], op=ALU.mult)
            out_writes(ft, r_aps[ft])


def build_nc(n_layers=L):
    nc = bacc.Bacc("TRN2", target_bir_lowering=False, debug=False,
                   num_devices=N_CORES)
    dt_ = nc.dram_tensor
    t = {}
    t["emb"] = dt_("emb_word", [32000, HD], F32, kind="ExternalInput").ap()
    t["ids"] = dt_("ids", [P, 12], I32, kind="ExternalInput").ap()
    t["pos"] = dt_("pos", [T_EXT, HD], F32, kind="ExternalInput").ap()
    t["eln_s"] = dt_("eln_s", [HD], F32, kind="ExternalInput").ap()
    t["eln_b"] = dt_("eln_b", [HD], F32, kind="ExternalInput").ap()
    t["wq"] = dt_("wq", [L, FT, P, FT, P], BF16, kind="ExternalInput").ap()
    t["wk"] = dt_("wk", [L, FT, P, FT, P], BF16, kind="ExternalInput").ap()
    t["wv"] = dt_("wv", [L, HD, HD], BF16, kind="ExternalInput").ap()
    t["wo"] = dt_("wo", [L, FT, P, FT, P], BF16, kind="ExternalInput").ap()
    t["w1"] = dt_("w1", [L, FFT, P, FT, P], BF16, kind="ExternalInput").ap()
    t["w2"] = dt_("w2", [L, FF, HD], BF16, kind="ExternalInput").ap()
    for nm in ["bq", "bk", "bo", "b2", "ls1", "lb1", "ls2", "lb2"]:
        t[nm] = dt_(nm, [L, P, FT], F32, kind="ExternalInput").ap()
    t["b1"] = dt_("b1", [L, P, FFT], F32, kind="ExternalInput").ap()
    t["sel"] = dt_("sel", [NH, FT * P], BF16, kind="ExternalInput").ap()
    t["ml"] = dt_("ml", [NCH, P, 512], BF16, kind="ExternalInput").ap()
    t["mr"] = dt_("mr", [NCH, P, 512], BF16, kind="ExternalInput").ap()
    t["halo_ids"] = dt_("halo_ids", [P, 12], I32, kind="ExternalInput").ap()
    t["out"] = dt_("out", [FT, P, T_OWN], F32, kind="ExternalOutput").ap()

    with tile.TileContext(nc) as tc:
        _build_body(nc, tc, n_layers, t)
    nc.compile()
    return nc


def _build_body(nc, tc, n_layers, t):
    from contextlib import ExitStack
    with ExitStack() as ctx:
        persist = ctx.enter_context(tc.tile_pool(name="persist", bufs=1))
        # residual stream x (own tokens, fp32) + bf16 shadow xb (with halos)
        x = [persist.tile([P, T_OWN], F32R, tag=f"x{ft}", name=f"x{ft}")
             for ft in range(FT)]
        xb = [persist.tile([P, T_EXT], BF16, tag=f"xb{ft}", name=f"xb{ft}")
              for ft in range(FT)]
        ml_sb = [persist.tile([P, 512], BF16, tag=f"ml{c}", name=f"ml{c}") for c in range(NCH)]
        mr_sb = [persist.tile([P, 512], BF16, tag=f"mr{c}", name=f"mr{c}") for c in range(NCH)]
        for c in range(NCH):
            nc.sync.dma_start(ml_sb[c][:], t["ml"][c])
            nc.sync.dma_start(mr_sb[c][:], t["mr"][c])
        ones_f = persist.tile([P, 1], F32, tag="ones_f", name="ones_f")
        nc.vector.memset(ones_f[:], 1.0)
        ones_r = persist.tile([P, 1], F32R, tag="ones_r", name="ones_r")
        nc.scalar.activation(ones_r[:], ones_f[:], AF.Identity)
        from concourse.masks import make_identity
        ident = persist.tile([P, P], F32, tag="ident", name="ident")
        make_identity(nc, ident[:])
        hid_sb = persist.tile([P, 12], I32, tag="hid", name="hid")
        nc.sync.dma_start(hid_sb[:], t["halo_ids"][:])
        sel_sb = persist.tile([NH, FT * P], BF16, tag="sel", name="sel")
        nc.sync.dma_start(sel_sb[:], t["sel"][:])
        bias_pool = ctx.enter_context(tc.tile_pool(name="biasp", bufs=2))
        eps_sb = persist.tile([P, 1], F32, tag="eps", name="eps")
        nc.vector.memset(eps_sb[:], EPS)

        # ---- embedding + LN -> x (own fp32) / xb (ext bf16) ----
        with tc.tile_pool(name="emb_sb", bufs=1) as esb, \
             tc.tile_pool(name="emb_sb2", bufs=2) as esb2, \
             tc.tile_pool(name="emb_ps", bufs=2, space="PSUM") as eps_p:
            ids_sb = esb.tile([P, 12], I32, tag="ids", name="ids")
            nc.sync.dma_start(ids_sb[:], t["ids"][:])
            s_bc = esb.tile([P, HD], F32, tag="sbc", name="sbc")
            nc.sync.dma_start(s_bc[:], bass.AP(
                tensor=t["eln_s"].tensor, offset=0, ap=[[0, P], [1, HD]]))
            b_bc = esb.tile([P, HD], F32, tag="bbc", name="bbc")
            nc.sync.dma_start(b_bc[:], bass.AP(
                tensor=t["eln_b"].tensor, offset=0, ap=[[0, P], [1, HD]]))
            e = [esb.tile([P, HD], F32, tag=f"e{tt}", name=f"e{tt}") for tt in range(12)]
            for tt in range(12):
                nc.gpsimd.indirect_dma_start(
                    out=e[tt][:], out_offset=None, in_=t["emb"][:],
                    in_offset=bass.IndirectOffsetOnAxis(
                        ap=ids_sb[:, tt:tt + 1], axis=0))
                p_sb = esb2.tile([P, HD], F32, tag="pos", name="pos")
                nc.sync.dma_start(p_sb[:], t["pos"][tt * P:(tt + 1) * P, :])
                nc.vector.tensor_tensor(e[tt][:], e[tt][:], p_sb[:], op=ALU.add)
                stats = esb2.tile([P, 3, nc.vector.BN_STATS_DIM], F32,
                                  tag="bst", name="bst")
                er = e[tt][:].rearrange("p (g d) -> p g d", g=3)
                for g in range(3):
                    nc.vector.bn_stats(stats[:, g, :], er[:, g, :])
                mv = esb2.tile([P, nc.vector.BN_AGGR_DIM], F32, tag="bag", name="bag")
                nc.vector.bn_aggr(mv[:], stats[:])
                sd = esb2.tile([P, 1], F32, tag="bsd", name="bsd")
                nc.scalar.activation(sd[:], mv[:, 1:2], AF.Sqrt, bias=eps_sb[:])
                rstd = esb2.tile([P, 1], F32, tag="brstd", name="brstd")
                nc.vector.reciprocal(rstd[:], sd[:])
                nc.vector.tensor_scalar(out=e[tt][:], in0=e[tt][:],
                                        scalar1=mv[:, 0:1], scalar2=rstd[:],
                                        op0=ALU.subtract, op1=ALU.mult)
                nc.vector.tensor_tensor(e[tt][:], e[tt][:], s_bc[:], op=ALU.mult)
                nc.vector.tensor_tensor(e[tt][:], e[tt][:], b_bc[:], op=ALU.add)
            for ft in range(FT):
                tr = eps_p.tile([P, T_EXT], F32, tag="tr", name="tr")
                for tt in range(12):
                    nc.tensor.transpose(tr[:, tt * P:(tt + 1) * P],
                                        e[tt][:, ft * P:(ft + 1) * P], ident[:])
                nc.scalar.activation(xb[ft][:], tr[:], AF.Identity)
                nc.scalar.activation(x[ft][:], tr[:, W:W + T_OWN], AF.Identity)

        for l in range(n_layers):
            _layer(nc, tc, t, l, x, xb, ml_sb, mr_sb, ones_r, eps_sb, hid_sb,
                   sel_sb, bias_pool, first=(l == 0),
                   exchange=(l < n_layers - 1))

        for ft in range(FT):
            nc.gpsimd.dma_start(t["out"][ft], x[ft][:])


def _attn_chunk(nc, c, kT, qT, v, ml_sb, mr_sb, o, den2, bsb, bps, bps2):
    for h in range(NH):
        ft, po = h // 2, (h % 2) * 64
        sps = bps.tile([P, 6 * W], F32, tag="sps", name="sps")
        for w in range(6):
            nc.tensor.matmul(
                sps[:, w * W:(w + 1) * W],
                kT[ft][po:po + 64, (c * 2 + w) * P:(c * 2 + w + 1) * P],
                qT[ft][po:po + 64, c * W:(c + 1) * W],
                start=True, stop=True)
        ex = bsb.tile([P, 6 * W], BF16, tag="ex", name="ex")
        nc.scalar.activation(ex[:], sps[:], AF.Exp)
        nc.vector.tensor_tensor(ex[:, 0:512], ex[:, 0:512],
                                ml_sb[c][:], op=ALU.mult)
        nc.vector.tensor_tensor(ex[:, 1024:1536], ex[:, 1024:1536],
                                mr_sb[c][:], op=ALU.mult)
        ops = bps2.tile([P, W], F32, tag="ops", name="ops")
        for w in range(6):
            nc.tensor.matmul(
                ops[0:65, :], v[c * 2 + w][:, h, :], ex[:, w * W:(w + 1) * W],
                start=(w == 0), stop=(w == 5))
        nc.vector.tensor_scalar_add(o[ft][po:po + 64, c * W:(c + 1) * W],
                                    ops[0:64, :], 0.0)
        # denominator row -> staging (partition 0) -> DMA into den2[h]
        st = bsb.tile([1, W], F32, tag="st", name="st")
        nc.vector.tensor_scalar_add(st[:], ops[64:65, :], 0.0)
        nc.sync.dma_start(den2[h:h + 1, c * W:(c + 1) * W], st[:])


def _layer(nc, tc, t, l, x, xb, ml_sb, mr_sb, ones_r, eps_sb, hid_sb,
           sel_sb, bias_pool, first, exchange):
    from contextlib import ExitStack
    with ExitStack() as ctx:
        lsb = ctx.enter_context(tc.tile_pool(name=f"lsb{l}", bufs=1))
        edram = None
        if exchange:
            edram = ctx.enter_context(
                tc.tile_pool(name=f"pe_dram{l}", bufs=1, space="DRAM"))

        # Allocation order matters: layer l+1's pool reuses these addresses,
        # so tiles written EARLY in a layer (qT/kT/o/weights) must sit on
        # addresses whose layer-l readers finish early (attention/O-proj),
        # while late-read tiles (y/yb/r2) live in the tail of the pool.
        qT = [lsb.tile([P, T_OWN], BF16, tag=f"qT{i}", name=f"qT{i}") for i in range(FT)]
        kT = [lsb.tile([P, T_EXT], BF16, tag=f"kT{i}", name=f"kT{i}") for i in range(FT)]
        o = [lsb.tile([P, T_OWN], BF16, tag=f"o{i}", name=f"o{i}") for i in range(FT)]
        wk_all = [lsb.tile([P, HD], BF16, tag=f"wk{i}", name=f"wk{i}")
                  for i in range(FT)]
        wv_all = [lsb.tile([P, HD], BF16, tag=f"wv{i}", name=f"wv{i}")
                  for i in range(FT)]
        den2 = lsb.tile([NH, T_OWN], F32, tag="den2", name="den2")
        rec = lsb.tile([NH, T_OWN], BF16, tag="rec", name="rec")
        y = [lsb.tile([P, T_OWN], F32R, tag=f"y{i}", name=f"y{i}") for i in range(FT)]
        yb = [lsb.tile([P, T_OWN], BF16, tag=f"yb{i}", name=f"yb{i}") for i in range(FT)]
        r2 = [lsb.tile([P, 512], F32R, tag=f"r2_{i}", name=f"r2_{i}") for i in range(FT)]
        r1 = y  # post-attention residual, normalized in place by LN1

        def bias_tile(name, n=FT):
            bt = bias_pool.tile([P, n], F32, tag=f"b_{name}", name=f"b_{name}")
            nc.sync.dma_start(bt[:], t[name][l])
            return bt
        bq_sb = bias_tile("bq"); bk_sb = bias_tile("bk"); bo_sb = bias_tile("bo")
        b1_sb = bias_tile("b1", FFT); b2_sb = bias_tile("b2")
        ls1_sb = bias_tile("ls1"); lb1_sb = bias_tile("lb1")
        ls2_sb = bias_tile("ls2"); lb2_sb = bias_tile("lb2")

        for i in range(FT):
            nc.sync.dma_start(wk_all[i][:], t["wk"][l, i])
            nc.sync.dma_start(wv_all[i][:], t["wv"][l, i * P:(i + 1) * P, :])

        with ExitStack() as vctx:
            vpool = vctx.enter_context(tc.tile_pool(name=f"vp{l}", bufs=1))
            v = [vpool.tile([P, NH, 65], BF16, tag=f"v{i}", name=f"v{i}")
                 for i in range(12)]

            # ---- phase A1: Q projection (own tokens, halo-independent) ----
            with tc.tile_pool(name=f"pa_sb{l}", bufs=4) as asb, \
                 tc.tile_pool(name=f"pa_ps{l}", bufs=4, space="PSUM") as aps, \
                 tc.tile_pool(name=f"pa_psv{l}", bufs=2, space="PSUM") as vps:
                for mt in range(FT):
                    wq_sb = asb.tile([P, HD], BF16, tag="wqs", name="wqs")
                    nc.sync.dma_start(wq_sb[:], t["wq"][l, mt])
                    for h2 in range(2):
                        ps = aps.tile([P, 512], F32, tag="pp", name="pp")
                        for kt in range(FT):
                            nc.tensor.matmul(
                                ps[:], wq_sb[:, kt * P:(kt + 1) * P],
                                xb[kt][:, W + h2 * 512:W + (h2 + 1) * 512],
                                start=(kt == 0), stop=(kt == FT - 1))
                        nc.scalar.activation(qT[mt][:, h2 * 512:(h2 + 1) * 512],
                                             ps[:], AF.Identity,
                                             bias=bq_sb[:, mt:mt + 1])
                # ---- A2: K projection, middle (own) columns ----
                for mt in range(FT):
                    for seg in range(2):
                        ps = aps.tile([P, 512], F32, tag="pp", name="pp")
                        for kt in range(FT):
                            nc.tensor.matmul(
                                ps[:], wk_all[mt][:, kt * P:(kt + 1) * P],
                                xb[kt][:, W + seg * 512:W + (seg + 1) * 512],
                                start=(kt == 0), stop=(kt == FT - 1))
                        nc.scalar.activation(
                            kT[mt][:, W + seg * 512:W + (seg + 1) * 512],
                            ps[:], AF.Identity, bias=bk_sb[:, mt:mt + 1])
                # ---- A3: V projection, own token tiles ----
                for tt in range(2, 10):
                    for hf in range(2):
                        ps = vps.tile([P, 384], F32, tag="ppv", name="ppv")
                        for kt in range(FT):
                            nc.tensor.matmul(
                                ps[:], xb[kt][:, tt * P:(tt + 1) * P],
                                wv_all[kt][:, hf * 384:(hf + 1) * 384],
                                start=(kt == 0), stop=(kt == FT - 1))
                        nc.scalar.activation(
                            v[tt][:, hf * 6:(hf + 1) * 6, 0:64],
                            ps[:].rearrange("p (h d) -> p h d", h=6), AF.Identity)
                    nc.vector.memset(v[tt][:, :, 64:65], 1.0)

            # ---- B-mid: attention chunks 1,2 (no halo dependency) ----
            with tc.tile_pool(name=f"pb_sb{l}", bufs=3) as bsb, \
                 tc.tile_pool(name=f"pb_ps{l}", bufs=2, space="PSUM") as bps, \
                 tc.tile_pool(name=f"pb_ps2{l}", bufs=2, space="PSUM") as bps2:
                for c in (1, 2):
                    _attn_chunk(nc, c, kT, qT, v, ml_sb, mr_sb, o, den2,
                                bsb, bps, bps2)

            # ---- A4/A5: K/V projections for halo columns ----
            # right halo first: its scatter (from the LN2h0-side AllGather of
            # the previous layer) completes earlier than the left one
            with tc.tile_pool(name=f"ph_ps{l}", bufs=4, space="PSUM") as hps, \
                 tc.tile_pool(name=f"ph_psv{l}", bufs=2, space="PSUM") as hvs:
                for side in (1, 0):
                    cs = slice(0, W) if side == 0 else slice(W + T_OWN, T_EXT)
                    for mt in range(FT):
                        ps = hps.tile([P, W], F32, tag="ph", name="ph")
                        for kt in range(FT):
                            nc.tensor.matmul(
                                ps[:], wk_all[mt][:, kt * P:(kt + 1) * P],
                                xb[kt][:, cs],
                                start=(kt == 0), stop=(kt == FT - 1))
                        nc.scalar.activation(kT[mt][:, cs], ps[:], AF.Identity,
                                             bias=bk_sb[:, mt:mt + 1])
                    for tt in ((10, 11) if side == 1 else (0, 1)):
                        for hf in range(2):
                            ps = hvs.tile([P, 384], F32, tag="phv", name="phv")
                            for kt in range(FT):
                                nc.tensor.matmul(
                                    ps[:], xb[kt][:, tt * P:(tt + 1) * P],
                                    wv_all[kt][:, hf * 384:(hf + 1) * 384],
                                    start=(kt == 0), stop=(kt == FT - 1))
                            nc.scalar.activation(
                                v[tt][:, hf * 6:(hf + 1) * 6, 0:64],
                                ps[:].rearrange("p (h d) -> p h d", h=6),
                                AF.Identity)
                        nc.vector.memset(v[tt][:, :, 64:65], 1.0)

            # ---- B-edge: attention chunks 3,0 ----
            with tc.tile_pool(name=f"pbe_sb{l}", bufs=3) as bsb, \
                 tc.tile_pool(name=f"pbe_ps{l}", bufs=2, space="PSUM") as bps, \
                 tc.tile_pool(name=f"pbe_ps2{l}", bufs=2, space="PSUM") as bps2:
                for c in (3, 0):
                    _attn_chunk(nc, c, kT, qT, v, ml_sb, mr_sb, o, den2,
                                bsb, bps, bps2)

        # ---- softmax normalization (batched) ----
        with nc.allow_low_precision("softmax denominators are O(1)-scaled"):
            nc.vector.reciprocal(rec[:], den2[:])
        with tc.tile_pool(name=f"pr{l}", bufs=2, space="PSUM") as rp:
            for ft in range(FT):
                # recb[m, q] = rec[2ft + (m >= 64), q] via selector matmul
                recb = rp.tile([P, T_OWN], F32, tag="recb", name="recb")
                for j in range(2):
                    nc.tensor.matmul(recb[:, j * 512:(j + 1) * 512],
                                     sel_sb[:, ft * P:(ft + 1) * P],
                                     rec[:, j * 512:(j + 1) * 512],
                                     start=True, stop=True)
                nc.vector.tensor_tensor(o[ft][:], o[ft][:], recb[:],
                                        op=ALU.mult)

        # ---- phase C: O-proj + residual (r1 <- x + O@Wo + bo) ----
        with tc.tile_pool(name=f"pc_sb{l}", bufs=3) as csb, \
             tc.tile_pool(name=f"pc_ps{l}", bufs=4, space="PSUM") as cps:
            for mt in range(FT):
                wo_sb = csb.tile([P, HD], BF16, tag="wos", name="wos")
                nc.sync.dma_start(wo_sb[:], t["wo"][l, mt])
                for h2 in range(2):
                    ps = cps.tile([P, 512], F32, tag="ppo", name="ppo")
                    for kt in range(FT):
                        nc.tensor.matmul(
                            ps[:], wo_sb[:, kt * P:(kt + 1) * P],
                            o[kt][:, h2 * 512:(h2 + 1) * 512],
                            start=(kt == 0), stop=(kt == FT - 1))
                    nc.vector.scalar_tensor_tensor(
                        out=r1[mt][:, h2 * 512:(h2 + 1) * 512], in0=ps[:],
                        scalar=bo_sb[:, mt:mt + 1],
                        in1=x[mt][:, h2 * 512:(h2 + 1) * 512],
                        op0=ALU.add, op1=ALU.add)

        # ---- LN1 (both halves at once): yb <- LN(r1)*s+b (bf16) ----
        def ln1_out(ft, src):
            nc.scalar.activation(yb[ft][:], src, AF.Identity,
                                 scale=ls1_sb[:, ft:ft + 1],
                                 bias=lb1_sb[:, ft:ft + 1])
        _ln_transposed(nc, tc, f"l1_{l}", [r1[ft][:] for ft in range(FT)],
                       ones_r, eps_sb, ln1_out, ncols=T_OWN)

        # ---- phase D: FFN + residual + LN2 -> x/xb ----
        for h2 in range(2):
            cs = slice(h2 * 512, (h2 + 1) * 512)
            with ExitStack() as dctx:
                dsb = dctx.enter_context(
                    tc.tile_pool(name=f"pd_sb{l}_{h2}", bufs=4))
                zps = dctx.enter_context(
                    tc.tile_pool(name=f"pd_psz{l}_{h2}", bufs=1, space="PSUM"))
                fps = dctx.enter_context(
                    tc.tile_pool(name=f"pd_psf{l}_{h2}", bufs=2, space="PSUM"))
                zp = [zps.tile([P, 512], F32, tag=f"z{mt}", name=f"z{mt}") for mt in range(FT)]
                for ms in range(FFT):
                    w1_sb = dsb.tile([P, HD], BF16, tag="w1s", name="w1s")
                    nc.sync.dma_start(w1_sb[:], t["w1"][l, ms])
                    fp = fps.tile([P, 512], F32, tag="fp", name="fp")
                    for kt in range(FT):
                        nc.tensor.matmul(fp[:], w1_sb[:, kt * P:(kt + 1) * P],
                                         yb[kt][:, cs], start=(kt == 0),
                                         stop=(kt == FT - 1))
                    f_sb = dsb.tile([P, 512], BF16, tag="fsb", name="fsb")
                    nc.scalar.activation(f_sb[:], fp[:], AF.Gelu,
                                         bias=b1_sb[:, ms:ms + 1])
                    w2_sb = dsb.tile([P, HD], BF16, tag="w2s", name="w2s")
                    nc.sync.dma_start(w2_sb[:],
                                      t["w2"][l, ms * P:(ms + 1) * P, :])
                    for mt in range(FT):
                        nc.tensor.matmul(zp[mt][:],
                                         w2_sb[:, mt * P:(mt + 1) * P],
                                         f_sb[:], start=(ms == 0),
                                         stop=(ms == FFT - 1))
                for mt in range(FT):
                    nc.vector.scalar_tensor_tensor(
                        out=r2[mt][:], in0=zp[mt][:],
                        scalar=b2_sb[:, mt:mt + 1], in1=yb[mt][:, cs],
                        op0=ALU.add, op1=ALU.add)

            def ln2_out(ft, src, h2=h2):
                off = h2 * 512
                nc.scalar.activation(xb[ft][:, W + off:W + off + 512], src,
                                     AF.Identity, scale=ls2_sb[:, ft:ft + 1],
                                     bias=lb2_sb[:, ft:ft + 1])
                nc.scalar.activation(x[ft][:, off:off + 512], src,
                                     AF.Identity, scale=ls2_sb[:, ft:ft + 1],
                                     bias=lb2_sb[:, ft:ft + 1])
            _ln_transposed(nc, tc, f"l2_{l}_{h2}",
                           [r2[ft][:] for ft in range(FT)],
                           ones_r, eps_sb, ln2_out)

            # Exchange left edges as soon as LN2 h0 lands: the AllGather of
            # left edges runs during FFN h1, and its scatter fills the RIGHT
            # halo (right neighbor's left edge) well before the next layer.
            if exchange and h2 == 0:
                b_in_l = edram.tile([FT, P, W], BF16, tag="binl", name="binl")
                b_out_l = edram.tile([4 * FT * P, W], BF16, tag="boutl",
                                     name="boutl")
                for ft in range(FT):
                    nc.sync.dma_start(b_in_l[ft], xb[ft][:, W:2 * W])
                nc.gpsimd.collective_compute(
                    "AllGather", ALU.bypass,
                    replica_groups=[[0, 1, 2, 3], [4, 5, 6, 7]],
                    ins=[b_in_l[:].opt()], outs=[b_out_l[:].opt()])
                for ft in range(FT):
                    nc.gpsimd.indirect_dma_start(
                        out=xb[ft][:, T_OWN + W:T_EXT], out_offset=None,
                        in_=b_out_l[:],
                        in_offset=bass.IndirectOffsetOnAxis(
                            ap=hid_sb[:, FT + ft:FT + ft + 1], axis=0))

        # ---- phase E: right-edge exchange (fills next layer's left halo) ----
        if exchange:
            b_in_r = edram.tile([FT, P, W], BF16, tag="binr", name="binr")
            b_out_r = edram.tile([4 * FT * P, W], BF16, tag="boutr",
                                 name="boutr")
            for ft in range(FT):
                nc.sync.dma_start(b_in_r[ft], xb[ft][:, T_OWN:T_OWN + W])
            nc.gpsimd.collective_compute(
                "AllGather", ALU.bypass,
                replica_groups=[[0, 1, 2, 3], [4, 5, 6, 7]],
                ins=[b_in_r[:].opt()], outs=[b_out_r[:].opt()])
            for ft in range(FT):
                nc.gpsimd.indirect_dma_start(
                    out=xb[ft][:, 0:W], out_offset=None, in_=b_out_r[:],
                    in_offset=bass.IndirectOffsetOnAxis(
                        ap=hid_sb[:, ft:ft + 1], axis=0))


# ---------------- host side ----------------

def _blocked(w, n_k, n_m):
    """[n_k*128, n_m*128] -> [n_m, 128, n_k, 128] (lhsT strips by out-tile)."""
    return np.ascontiguousarray(
        w.reshape(n_k, P, n_m, P).transpose(2, 1, 0, 3))


def _bias_lay(b, n):
    return np.ascontiguousarray(b.reshape(n, P).T)


def prepare(inputs):
    """Build per-core in_maps from full inputs."""
    ids_full = np.asarray(inputs["input_ids"]).astype(np.int32)
    am = np.asarray(inputs["attention_mask"]).astype(np.int32)
    emb_word = np.asarray(inputs["emb_word"], dtype=np.float32)
    emb_pos = np.asarray(inputs["emb_pos"], dtype=np.float32)
    Wq = np.asarray(inputs["Wq"], np.float32) / np.sqrt(DH)
    bq = np.asarray(inputs["bq"], np.float32) / np.sqrt(DH)
    Wk = np.asarray(inputs["Wk"], np.float32)
    bk = np.asarray(inputs["bk"], np.float32)
    Wv = np.asarray(inputs["Wv"], np.float32)
    bv = np.asarray(inputs["bv"], np.float32)
    Wo = np.asarray(inputs["Wo"], np.float32)
    bo = np.asarray(inputs["bo"], np.float32)
    W1 = np.asarray(inputs["W1"], np.float32)
    b1 = np.asarray(inputs["b1"], np.float32)
    W2 = np.asarray(inputs["W2"], np.float32)
    b2 = np.asarray(inputs["b2"], np.float32)
    assert np.all(am == 1), "general attention_mask needs mid-tile masks too"
    bf = ml_dtypes.bfloat16

    shared = {
        "emb_word": emb_word,
        "eln_s": np.asarray(inputs["emb_ln_s"], np.float32),
        "eln_b": np.asarray(inputs["emb_ln_b"], np.float32),
        "wq": np.stack([_blocked(Wq[i], FT, FT) for i in range(L)]).astype(bf),
        "wk": np.stack([_blocked(Wk[i], FT, FT) for i in range(L)]).astype(bf),
        "wv": Wv.astype(bf),
        "wo": np.stack([_blocked(Wo[i], FT, FT) for i in range(L)]).astype(bf),
        "w1": np.stack([_blocked(W1[i], FT, FFT) for i in range(L)]).astype(bf),
        "w2": W2.astype(bf),
        "bq": np.stack([_bias_lay(bq[i], FT) for i in range(L)]),
        "bk": np.stack([_bias_lay(bk[i], FT) for i in range(L)]),
        "bo": np.stack([_bias_lay(bv[i] @ Wo[i] + bo[i], FT)
                        for i in range(L)]),
        "b1": np.stack([_bias_lay(b1[i], FFT) for i in range(L)]),
        "b2": np.stack([_bias_lay(b2[i], FT) for i in range(L)]),
        "ls1": np.stack([_bias_lay(np.asarray(inputs["ln1_s"], np.float32)[i],
                                   FT) for i in range(L)]),
        "lb1": np.stack([_bias_lay(np.asarray(inputs["ln1_b"], np.float32)[i],
                                   FT) for i in range(L)]),
        "ls2": np.stack([_bias_lay(np.asarray(inputs["ln2_s"], np.float32)[i],
                                   FT) for i in range(L)]),
        "lb2": np.stack([_bias_lay(np.asarray(inputs["ln2_b"], np.float32)[i],
                                   FT) for i in range(L)]),
    }
    sel = np.zeros((NH, FT * P), np.float32)
    for ft in range(FT):
        sel[2 * ft, ft * P:ft * P + 64] = 1.0
        sel[2 * ft + 1, ft * P + 64:(ft + 1) * P] = 1.0
    shared["sel"] = sel.astype(bf)

    in_maps = []
    i_idx = np.arange(W)
    for core in range(N_CORES):
        b, sb = core // 4, core % 4
        s0 = sb * T_OWN
        ext_pos = np.clip(np.arange(s0 - W, s0 + T_OWN + W), 0, S - 1)
        m = dict(shared)
        m["ids"] = np.ascontiguousarray(
            ids_full[b, ext_pos].reshape(12, P).T)
        m["pos"] = np.ascontiguousarray(emb_pos[ext_pos])
        # masks: global chunk gc, window key j in [0,768), query i in [0,256):
        #   key_abs = gc*W - W + j ; allowed = |j - W - i| <= W
        #             & 0 <= key_abs < S & attention_mask[b, key_abs]
        mlm = np.zeros((NCH, P, 512), np.float32)
        mrm = np.zeros((NCH, P, 512), np.float32)
        for c in range(NCH):
            gc = sb * NCH + c
            for kt2 in range(2):
                for mm_, j0 in ((mlm, 0), (mrm, 512)):
                    j = j0 + kt2 * P + np.arange(P)[:, None]
                    key_abs = gc * W - W + j
                    ok = (np.abs(j - W - i_idx[None, :]) <= W)
                    ok &= (key_abs >= 0) & (key_abs < S)
                    ok &= am[b, np.clip(key_abs, 0, S - 1)] > 0
                    mm_[c, :, kt2 * W:(kt2 + 1) * W] = ok
        m["ml"] = mlm.astype(bf)
        m["mr"] = mrm.astype(bf)
        # halo row ids: cols 0-5 index b_out_r (right edges; left halo comes
        # from left neighbor's right edge), cols 6-11 index b_out_l (left
        # edges; right halo from right neighbor's left edge). Both tables are
        # [4, FT, 128, W] row-major. Edge cores fall back to their own rows
        # (bounded values; masked out of the attention anyway).
        hid = np.zeros((2, FT, P), np.int64)
        for side in range(2):
            nb = sb - 1 if side == 0 else sb + 1
            src = nb if 0 <= nb <= 3 else sb
            for ft in range(FT):
                hid[side, ft] = (src * FT + ft) * P + np.arange(P)
        m["halo_ids"] = np.ascontiguousarray(
            hid.reshape(12, P).T.astype(np.int32))
        in_maps.append(m)
    return in_maps


_NC_CACHE = {}


def get_nc(n_layers=L):
    if n_layers not in _NC_CACHE:
        _NC_CACHE[n_layers] = build_nc(n_layers)
    return _NC_CACHE[n_layers]


def run(inputs, n_layers=L, trace=False):
    nc = get_nc(n_layers)
    in_maps = prepare(inputs)
    res = bass_utils.run_bass_kernel_spmd(
        nc, in_maps, core_ids=list(range(N_CORES)), trace=trace)
    outs = np.empty((B, S, HD), np.float32)
    for core in range(N_CORES):
        b, sb = core // 4, core % 4
        ot = res.results[core]["out"]  # [FT, 128, T_OWN]
        outs[b, sb * T_OWN:(sb + 1) * T_OWN] = ot.reshape(HD, T_OWN).T
    return outs, res


def kernel(**inputs) -> np.ndarray:
    out, _ = run(inputs)
    return out


# revision 44
# speedup vs baseline: 1.0627x; 1.0266x over previous
"""Longformer layer stack (4 layers, sliding-window attention) on 8 TRN2 cores.

Sharding: data-parallel over batch (2) x sequence-parallel (4 blocks of 1024
tokens). Each core computes its sequence block; the banded attention needs a
W=256 token halo, exchanged between neighboring blocks with an AllGather after
each layer (layers 0-2), overlapped with the next layer's halo-independent
work (Q proj, middle K/V, attention chunks 1-2).

Residual stream x kept transposed ([dmodel, tokens]) in fp32; a bf16 shadow xb
(with halos) feeds all matmuls. All matmul operands are bf16 except the
LayerNorm sum-reductions (fp32r). Softmax normalization is deferred: raw
exp-sums accumulate into o, denominators batch into one reciprocal per layer.
"""
import sys

sys.path.insert(0, '/opt/trn_rl_repo')

import numpy as np
import ml_dtypes

import concourse.bass as bass
import concourse.mybir as mybir
import concourse.tile as tile
from concourse import bacc
from concourse import bass_utils

F32 = mybir.dt.float32
F32R = mybir.dt.float32r
BF16 = mybir.dt.bfloat16
I32 = mybir.dt.int32
AF = mybir.ActivationFunctionType
ALU = mybir.AluOpType

NH = 12          # heads
DH = 64          # head dim
HD = 768         # model dim
FF = 3072        # ffn dim
W = 256          # one-sided window
L = 4            # layers
B = 2
S = 4096
EPS = 1e-12
N_CORES = 8
T_OWN = 1024     # tokens per core
T_EXT = 1536     # with halos
FT = 6           # model-dim 128-tiles
FFT = 24         # ffn-dim 128-tiles
NCH = 4          # local chunks of 256 queries
P = 128


def _ln_sums(nc, sb, sqp, pp, tag, r_aps, ones_r):
    """Emit the PE sum/sumsq reductions; returns tiles for _ln_chain."""
    sumx = pp.tile([1, 512], F32, tag="sumx", name=f"sumx_{tag}")
    sumsq = pp.tile([1, 512], F32, tag="sumsq", name=f"sumsq_{tag}")
    for ft in range(FT):
        sq = sqp.tile([P, 512], F32R, tag="sq", name=f"sq_{tag}")
        nc.scalar.activation(sq[:], r_aps[ft], AF.Square)
        nc.tensor.matmul(sumx[0:1, :], ones_r[:, 0:1], r_aps[ft],
                         start=(ft == 0), stop=(ft == FT - 1))
        nc.tensor.matmul(sumsq[0:1, :], ones_r[:, 0:1], sq[:],
                         start=(ft == 0), stop=(ft == FT - 1))
    return sumx, sumsq


def _ln_chain(nc, sb, tag, sumx, sumsq, r_aps, eps_sb, out_writes):
    """Stats + normalize (in place) + out_writes; rstd via exp(-ln(sd))."""
    # stat = [mu || rstd]; rstd = exp(-0.5*ln(var+eps)) avoids the slow
    # DVE reciprocal custom op.
    stat = sb.tile([1, 1024], F32, tag="stat", name=f"stat_{tag}")
    nc.scalar.activation(stat[0:1, 0:512], sumx[:], AF.Identity,
                         scale=1.0 / HD)
    musq = sb.tile([1, 512], F32, tag="musq", name=f"musq_{tag}")
    nc.scalar.activation(musq[:], stat[0:1, 0:512], AF.Square)
    var = sb.tile([1, 512], F32, tag="var", name=f"var_{tag}")
    nc.vector.scalar_tensor_tensor(out=var[:], in0=sumsq[:],
                                   scalar=1.0 / HD, in1=musq[:],
                                   op0=ALU.mult, op1=ALU.subtract)
    lv = sb.tile([1, 512], F32, tag="lv", name=f"lv_{tag}")
    nc.scalar.activation(lv[:], var[:], AF.Ln, bias=eps_sb[0:1, :])
    nc.scalar.activation(stat[0:1, 512:1024], lv[:], AF.Exp, scale=-0.5)
    stat_b = sb.tile([P, 1024], F32, tag="statb", name=f"statb_{tag}")
    nc.gpsimd.partition_broadcast(stat_b[:], stat[:], channels=P)
    for ft in range(FT):
        nc.vector.tensor_tensor(r_aps[ft], r_aps[ft], stat_b[:, 0:512],
                                op=ALU.subtract)
        nc.vector.tensor_tensor(r_aps[ft], r_aps[ft], stat_b[:, 512:1024],
                                op=ALU.mult)
        out_writes(ft, r_aps[ft])


def _ln_transposed(nc, tc, tag, r_aps, ones_r, eps_sb, out_writes):
    """LayerNorm over the partition (feature) axis of transposed [P,512]
    tiles, normalized IN PLACE up to out_writes(ft, src)."""
    with tc.tile_pool(name=f"lnsb_{tag}", bufs=1) as sb, \
         tc.tile_pool(name=f"lnsq_{tag}", bufs=2) as sqp, \
         tc.tile_pool(name=f"lnps_{tag}", bufs=1, space="PSUM") as pp:
        sumx, sumsq = _ln_sums(nc, sb, sqp, pp, tag, r_aps, ones_r)
        _ln_chain(nc, sb, tag, sumx, sumsq, r_aps, eps_sb, out_writes)


def build_nc(n_layers=L):
    nc = bacc.Bacc("TRN2", target_bir_lowering=False, debug=False,
                   num_devices=N_CORES)
    dt_ = nc.dram_tensor
    t = {}
    t["emb"] = dt_("emb_word", [32000, HD], F32, kind="ExternalInput").ap()
    t["ids"] = dt_("ids", [P, 12], I32, kind="ExternalInput").ap()
    t["pos"] = dt_("pos", [T_EXT, HD], F32, kind="ExternalInput").ap()
    t["eln_s"] = dt_("eln_s", [HD], F32, kind="ExternalInput").ap()
    t["eln_b"] = dt_("eln_b", [HD], F32, kind="ExternalInput").ap()
    t["wq"] = dt_("wq", [L, FT, P, FT, P], BF16, kind="ExternalInput").ap()
    t["wk"] = dt_("wk", [L, FT, P, FT, P], BF16, kind="ExternalInput").ap()
    t["wv"] = dt_("wv", [L, HD, HD], BF16, kind="ExternalInput").ap()
    t["wo"] = dt_("wo", [L, FT, P, FT, P], BF16, kind="ExternalInput").ap()
    t["w1"] = dt_("w1", [L, FFT, P, FT, P], BF16, kind="ExternalInput").ap()
    t["w2"] = dt_("w2", [L, FF, HD], BF16, kind="ExternalInput").ap()
    for nm in ["bq", "bk", "bo", "b2", "ls1", "lb1", "ls2", "lb2"]:
        t[nm] = dt_(nm, [L, P, FT], F32, kind="ExternalInput").ap()
    t["b1"] = dt_("b1", [L, P, FFT], F32, kind="ExternalInput").ap()
    t["sel"] = dt_("sel", [NH, FT * P], BF16, kind="ExternalInput").ap()
    t["ml"] = dt_("ml", [NCH, P, 512], BF16, kind="ExternalInput").ap()
    t["mr"] = dt_("mr", [NCH, P, 512], BF16, kind="ExternalInput").ap()
    t["halo_ids"] = dt_("halo_ids", [P, 12], I32, kind="ExternalInput").ap()
    t["out"] = dt_("out", [FT, P, T_OWN], F32, kind="ExternalOutput").ap()

    with tile.TileContext(nc) as tc:
        _build_body(nc, tc, n_layers, t)
    nc.compile()
    return nc


def _build_body(nc, tc, n_layers, t):
    from contextlib import ExitStack
    with ExitStack() as ctx:
        persist = ctx.enter_context(tc.tile_pool(name="persist", bufs=1))
        # residual stream x (own tokens, fp32) + bf16 shadow xb (with halos)
        x = [persist.tile([P, T_OWN], F32R, tag=f"x{ft}", name=f"x{ft}")
             for ft in range(FT)]
        xb = [persist.tile([P, T_EXT], BF16, tag=f"xb{ft}", name=f"xb{ft}")
              for ft in range(FT)]
        ml_sb = [persist.tile([P, 512], BF16, tag=f"ml{c}", name=f"ml{c}") for c in range(NCH)]
        mr_sb = [persist.tile([P, 512], BF16, tag=f"mr{c}", name=f"mr{c}") for c in range(NCH)]
        for c in range(NCH):
            nc.sync.dma_start(ml_sb[c][:], t["ml"][c])
            nc.sync.dma_start(mr_sb[c][:], t["mr"][c])
        ones_f = persist.tile([P, 1], F32, tag="ones_f", name="ones_f")
        nc.vector.memset(ones_f[:], 1.0)
        ones_r = persist.tile([P, 1], F32R, tag="ones_r", name="ones_r")
        nc.scalar.activation(ones_r[:], ones_f[:], AF.Identity)
        from concourse.masks import make_identity
        ident = persist.tile([P, P], F32, tag="ident", name="ident")
        make_identity(nc, ident[:])
        hid_sb = persist.tile([P, 12], I32, tag="hid", name="hid")
        nc.sync.dma_start(hid_sb[:], t["halo_ids"][:])
        sel_sb = persist.tile([NH, FT * P], BF16, tag="sel", name="sel")
        nc.sync.dma_start(sel_sb[:], t["sel"][:])
        bias_pool = ctx.enter_context(tc.tile_pool(name="biasp", bufs=2))
        eps_sb = persist.tile([P, 1], F32, tag="eps", name="eps")
        nc.vector.memset(eps_sb[:], EPS)

        # ---- embedding + LN -> x (own fp32) / xb (ext bf16) ----
        with tc.tile_pool(name="emb_sb", bufs=1) as esb, \
             tc.tile_pool(name="emb_sb2", bufs=2) as esb2, \
             tc.tile_pool(name="emb_ps", bufs=2, space="PSUM") as eps_p:
            ids_sb = esb.tile([P, 12], I32, tag="ids", name="ids")
            nc.sync.dma_start(ids_sb[:], t["ids"][:])
            s_bc = esb.tile([P, HD], F32, tag="sbc", name="sbc")
            nc.sync.dma_start(s_bc[:], bass.AP(
                tensor=t["eln_s"].tensor, offset=0, ap=[[0, P], [1, HD]]))
            b_bc = esb.tile([P, HD], F32, tag="bbc", name="bbc")
            nc.sync.dma_start(b_bc[:], bass.AP(
                tensor=t["eln_b"].tensor, offset=0, ap=[[0, P], [1, HD]]))
            e = [esb.tile([P, HD], F32, tag=f"e{tt}", name=f"e{tt}") for tt in range(12)]
            for tt in range(12):
                nc.gpsimd.indirect_dma_start(
                    out=e[tt][:], out_offset=None, in_=t["emb"][:],
                    in_offset=bass.IndirectOffsetOnAxis(
                        ap=ids_sb[:, tt:tt + 1], axis=0))
                p_sb = esb2.tile([P, HD], F32, tag="pos", name="pos")
                nc.sync.dma_start(p_sb[:], t["pos"][tt * P:(tt + 1) * P, :])
                nc.vector.tensor_tensor(e[tt][:], e[tt][:], p_sb[:], op=ALU.add)
                stats = esb2.tile([P, 3, nc.vector.BN_STATS_DIM], F32,
                                  tag="bst", name="bst")
                er = e[tt][:].rearrange("p (g d) -> p g d", g=3)
                for g in range(3):
                    nc.vector.bn_stats(stats[:, g, :], er[:, g, :])
                mv = esb2.tile([P, nc.vector.BN_AGGR_DIM], F32, tag="bag", name="bag")
                nc.vector.bn_aggr(mv[:], stats[:])
                sd = esb2.tile([P, 1], F32, tag="bsd", name="bsd")
                nc.scalar.activation(sd[:], mv[:, 1:2], AF.Sqrt, bias=eps_sb[:])
                rstd = esb2.tile([P, 1], F32, tag="brstd", name="brstd")
                nc.vector.reciprocal(rstd[:], sd[:])
                nc.vector.tensor_scalar(out=e[tt][:], in0=e[tt][:],
                                        scalar1=mv[:, 0:1], scalar2=rstd[:],
                                        op0=ALU.subtract, op1=ALU.mult)
                nc.vector.tensor_tensor(e[tt][:], e[tt][:], s_bc[:], op=ALU.mult)
                nc.vector.tensor_tensor(e[tt][:], e[tt][:], b_bc[:], op=ALU.add)
            for ft in range(FT):
                tr = eps_p.tile([P, T_EXT], F32, tag="tr", name="tr")
                for tt in range(12):
                    nc.tensor.transpose(tr[:, tt * P:(tt + 1) * P],
                                        e[tt][:, ft * P:(ft + 1) * P], ident[:])
                nc.scalar.activation(xb[ft][:], tr[:], AF.Identity)
                nc.scalar.activation(x[ft][:], tr[:, W:W + T_OWN], AF.Identity)

        for l in range(n_layers):
            _layer(nc, tc, t, l, x, xb, ml_sb, mr_sb, ones_r, eps_sb, hid_sb,
                   sel_sb, bias_pool, first=(l == 0),
                   exchange=(l < n_layers - 1))

        for ft in range(FT):
            nc.gpsimd.dma_start(t["out"][ft], x[ft][:])


def _attn_chunk(nc, c, kT, qT, v, ml_sb, mr_sb, o, den2, bsb, bps, bps2):
    for h in range(NH):
        ft, po = h // 2, (h % 2) * 64
        sps = bps.tile([P, 6 * W], F32, tag="sps", name="sps")
        for w in range(6):
            nc.tensor.matmul(
                sps[:, w * W:(w + 1) * W],
                kT[ft][po:po + 64, (c * 2 + w) * P:(c * 2 + w + 1) * P],
                qT[ft][po:po + 64, c * W:(c + 1) * W],
                start=True, stop=True)
        ex = bsb.tile([P, 6 * W], BF16, tag="ex", name="ex")
        nc.scalar.activation(ex[:], sps[:], AF.Exp)
        nc.vector.tensor_tensor(ex[:, 0:512], ex[:, 0:512],
                                ml_sb[c][:], op=ALU.mult)
        nc.vector.tensor_tensor(ex[:, 1024:1536], ex[:, 1024:1536],
                                mr_sb[c][:], op=ALU.mult)
        ops = bps2.tile([P, W], F32, tag="ops", name="ops")
        for w in range(6):
            nc.tensor.matmul(
                ops[0:65, :], v[c * 2 + w][:, h, :], ex[:, w * W:(w + 1) * W],
                start=(w == 0), stop=(w == 5))
        nc.vector.tensor_scalar_add(o[ft][po:po + 64, c * W:(c + 1) * W],
                                    ops[0:64, :], 0.0)
        # denominator row -> staging (partition 0) -> DMA into den2[h]
        st = bsb.tile([1, W], F32, tag="st", name="st")
        nc.vector.tensor_scalar_add(st[:], ops[64:65, :], 0.0)
        nc.sync.dma_start(den2[h:h + 1, c * W:(c + 1) * W], st[:])


def _layer(nc, tc, t, l, x, xb, ml_sb, mr_sb, ones_r, eps_sb, hid_sb,
           sel_sb, bias_pool, first, exchange):
    from contextlib import ExitStack
    with ExitStack() as ctx:
        lsb = ctx.enter_context(tc.tile_pool(name=f"lsb{l}", bufs=1))
        edram = None
        if exchange:
            edram = ctx.enter_context(
                tc.tile_pool(name=f"pe_dram{l}", bufs=1, space="DRAM"))

        # Allocation order matters: layer l+1's pool reuses these addresses,
        # so tiles written EARLY in a layer (qT/kT/o/weights) must sit on
        # addresses whose layer-l readers finish early (attention/O-proj),
        # while late-read tiles (y/yb/r2) live in the tail of the pool.
        qT = [lsb.tile([P, T_OWN], BF16, tag=f"qT{i}", name=f"qT{i}") for i in range(FT)]
        kT = [lsb.tile([P, T_EXT], BF16, tag=f"kT{i}", name=f"kT{i}") for i in range(FT)]
        o = [lsb.tile([P, T_OWN], BF16, tag=f"o{i}", name=f"o{i}") for i in range(FT)]
        wk_all = [lsb.tile([P, HD], BF16, tag=f"wk{i}", name=f"wk{i}")
                  for i in range(FT)]
        wv_all = [lsb.tile([P, HD], BF16, tag=f"wv{i}", name=f"wv{i}")
                  for i in range(FT)]
        den2 = lsb.tile([NH, T_OWN], F32, tag="den2", name="den2")
        rec = lsb.tile([NH, T_OWN], BF16, tag="rec", name="rec")
        y = [lsb.tile([P, T_OWN], F32R, tag=f"y{i}", name=f"y{i}") for i in range(FT)]
        yb = [lsb.tile([P, T_OWN], BF16, tag=f"yb{i}", name=f"yb{i}") for i in range(FT)]
        r2 = [lsb.tile([P, 512], F32R, tag=f"r2_{i}", name=f"r2_{i}") for i in range(FT)]
        r1 = y  # post-attention residual, normalized in place by LN1

        def bias_tile(name, n=FT):
            bt = bias_pool.tile([P, n], F32, tag=f"b_{name}", name=f"b_{name}")
            nc.sync.dma_start(bt[:], t[name][l])
            return bt
        bq_sb = bias_tile("bq"); bk_sb = bias_tile("bk"); bo_sb = bias_tile("bo")
        b1_sb = bias_tile("b1", FFT); b2_sb = bias_tile("b2")
        ls1_sb = bias_tile("ls1"); lb1_sb = bias_tile("lb1")
        ls2_sb = bias_tile("ls2"); lb2_sb = bias_tile("lb2")

        for i in range(FT):
            nc.sync.dma_start(wk_all[i][:], t["wk"][l, i])
            nc.sync.dma_start(wv_all[i][:], t["wv"][l, i * P:(i + 1) * P, :])

        with ExitStack() as vctx:
            vpool = vctx.enter_context(tc.tile_pool(name=f"vp{l}", bufs=1))
            v = [vpool.tile([P, NH, 65], BF16, tag=f"v{i}", name=f"v{i}")
                 for i in range(12)]

            # ---- phase A: Q/K-middle/V-own projections. All h0-column work
            # precedes h1-column work so the PE never queues behind the
            # previous layer's LN2 h1 chain.
            with tc.tile_pool(name=f"pa_sb{l}", bufs=4) as asb, \
                 tc.tile_pool(name=f"pa_ps{l}", bufs=4, space="PSUM") as aps, \
                 tc.tile_pool(name=f"pa_psv{l}", bufs=2, space="PSUM") as vps:
                for h2 in range(2):
                    for mt in range(FT):  # Q half
                        wq_sb = asb.tile([P, HD], BF16, tag="wqs", name="wqs")
                        nc.sync.dma_start(wq_sb[:], t["wq"][l, mt])
                        ps = aps.tile([P, 512], F32, tag="pp", name="pp")
                        for kt in range(FT):
                            nc.tensor.matmul(
                                ps[:], wq_sb[:, kt * P:(kt + 1) * P],
                                xb[kt][:, W + h2 * 512:W + (h2 + 1) * 512],
                                start=(kt == 0), stop=(kt == FT - 1))
                        nc.scalar.activation(qT[mt][:, h2 * 512:(h2 + 1) * 512],
                                             ps[:], AF.Identity,
                                             bias=bq_sb[:, mt:mt + 1])
                    for mt in range(FT):  # K middle half
                        ps = aps.tile([P, 512], F32, tag="pp", name="pp")
                        for kt in range(FT):
                            nc.tensor.matmul(
                                ps[:], wk_all[mt][:, kt * P:(kt + 1) * P],
                                xb[kt][:, W + h2 * 512:W + (h2 + 1) * 512],
                                start=(kt == 0), stop=(kt == FT - 1))
                        nc.scalar.activation(
                            kT[mt][:, W + h2 * 512:W + (h2 + 1) * 512],
                            ps[:], AF.Identity, bias=bk_sb[:, mt:mt + 1])
                    for tt in range(2 + 4 * h2, 6 + 4 * h2):  # V own half
                        for hf in range(2):
                            ps = vps.tile([P, 384], F32, tag="ppv", name="ppv")
                            for kt in range(FT):
                                nc.tensor.matmul(
                                    ps[:], xb[kt][:, tt * P:(tt + 1) * P],
                                    wv_all[kt][:, hf * 384:(hf + 1) * 384],
                                    start=(kt == 0), stop=(kt == FT - 1))
                            nc.scalar.activation(
                                v[tt][:, hf * 6:(hf + 1) * 6, 0:64],
                                ps[:].rearrange("p (h d) -> p h d", h=6),
                                AF.Identity)
                        nc.vector.memset(v[tt][:, :, 64:65], 1.0)

            # ---- B-mid: attention chunks 1,2 (no halo dependency) ----
            with tc.tile_pool(name=f"pb_sb{l}", bufs=3) as bsb, \
                 tc.tile_pool(name=f"pb_ps{l}", bufs=2, space="PSUM") as bps, \
                 tc.tile_pool(name=f"pb_ps2{l}", bufs=2, space="PSUM") as bps2:
                for c in (1, 2):
                    _attn_chunk(nc, c, kT, qT, v, ml_sb, mr_sb, o, den2,
                                bsb, bps, bps2)

            # ---- A4/A5: K/V projections for halo columns ----
            # right halo first: its scatter (from the LN2h0-side AllGather of
            # the previous layer) completes earlier than the left one
            with tc.tile_pool(name=f"ph_ps{l}", bufs=4, space="PSUM") as hps, \
                 tc.tile_pool(name=f"ph_psv{l}", bufs=2, space="PSUM") as hvs:
                for side in (1, 0):
                    cs = slice(0, W) if side == 0 else slice(W + T_OWN, T_EXT)
                    for mt in range(FT):
                        ps = hps.tile([P, W], F32, tag="ph", name="ph")
                        for kt in range(FT):
                            nc.tensor.matmul(
                                ps[:], wk_all[mt][:, kt * P:(kt + 1) * P],
                                xb[kt][:, cs],
                                start=(kt == 0), stop=(kt == FT - 1))
                        nc.scalar.activation(kT[mt][:, cs], ps[:], AF.Identity,
                                             bias=bk_sb[:, mt:mt + 1])
                    for tt in ((10, 11) if side == 1 else (0, 1)):
                        for hf in range(2):
                            ps = hvs.tile([P, 384], F32, tag="phv", name="phv")
                            for kt in range(FT):
                                nc.tensor.matmul(
                                    ps[:], xb[kt][:, tt * P:(tt + 1) * P],
                                    wv_all[kt][:, hf * 384:(hf + 1) * 384],
                                    start=(kt == 0), stop=(kt == FT - 1))
                            nc.scalar.activation(
                                v[tt][:, hf * 6:(hf + 1) * 6, 0:64],
                                ps[:].rearrange("p (h d) -> p h d", h=6),
                                AF.Identity)
                        nc.vector.memset(v[tt][:, :, 64:65], 1.0)

            # ---- B-edge: attention chunks 3,0 ----
            with tc.tile_pool(name=f"pbe_sb{l}", bufs=3) as bsb, \
                 tc.tile_pool(name=f"pbe_ps{l}", bufs=2, space="PSUM") as bps, \
                 tc.tile_pool(name=f"pbe_ps2{l}", bufs=2, space="PSUM") as bps2:
                for c in (3, 0):
                    _attn_chunk(nc, c, kT, qT, v, ml_sb, mr_sb, o, den2,
                                bsb, bps, bps2)

        # ---- softmax normalization (batched): rec = exp(-ln(den)) ----
        nc.scalar.activation(den2[:], den2[:], AF.Ln)
        nc.scalar.activation(rec[:], den2[:], AF.Exp, scale=-1.0)
        with tc.tile_pool(name=f"pr{l}", bufs=2, space="PSUM") as rp:
            for ft in range(FT):
                # recb[m, q] = rec[2ft + (m >= 64), q] via selector matmul
                recb = rp.tile([P, T_OWN], F32, tag="recb", name="recb")
                for j in range(2):
                    nc.tensor.matmul(recb[:, j * 512:(j + 1) * 512],
                                     sel_sb[:, ft * P:(ft + 1) * P],
                                     rec[:, j * 512:(j + 1) * 512],
                                     start=True, stop=True)
                nc.vector.tensor_tensor(o[ft][:], o[ft][:], recb[:],
                                        op=ALU.mult)

        # ---- phase C: O-proj + residual (r1 <- x + O@Wo + bo) ----
        with tc.tile_pool(name=f"pc_sb{l}", bufs=3) as csb, \
             tc.tile_pool(name=f"pc_ps{l}", bufs=4, space="PSUM") as cps:
            for mt in range(FT):
                wo_sb = csb.tile([P, HD], BF16, tag="wos", name="wos")
                nc.sync.dma_start(wo_sb[:], t["wo"][l, mt])
                for h2 in range(2):
                    ps = cps.tile([P, 512], F32, tag="ppo", name="ppo")
                    for kt in range(FT):
                        nc.tensor.matmul(
                            ps[:], wo_sb[:, kt * P:(kt + 1) * P],
                            o[kt][:, h2 * 512:(h2 + 1) * 512],
                            start=(kt == 0), stop=(kt == FT - 1))
                    nc.vector.scalar_tensor_tensor(
                        out=r1[mt][:, h2 * 512:(h2 + 1) * 512], in0=ps[:],
                        scalar=bo_sb[:, mt:mt + 1],
                        in1=x[mt][:, h2 * 512:(h2 + 1) * 512],
                        op0=ALU.add, op1=ALU.add)

        # ---- LN1: yb <- LN(r1)*s+b (bf16). Both halves' PE sums are
        # emitted before either DVE chain, so chain h1 hides under FFN h0.
        def ln1_out(h2):
            def write(ft, src):
                nc.scalar.activation(yb[ft][:, h2 * 512:(h2 + 1) * 512], src,
                                     AF.Identity, scale=ls1_sb[:, ft:ft + 1],
                                     bias=lb1_sb[:, ft:ft + 1])
            return write
        with tc.tile_pool(name=f"l1sb_{l}", bufs=2) as l1sb, \
             tc.tile_pool(name=f"l1sq_{l}", bufs=2) as l1sq, \
             tc.tile_pool(name=f"l1ps_{l}", bufs=2, space="PSUM") as l1ps:
            halves = [[r1[ft][:, h2 * 512:(h2 + 1) * 512] for ft in range(FT)]
                      for h2 in range(2)]
            sums = [_ln_sums(nc, l1sb, l1sq, l1ps, f"l1_{l}_{h2}",
                             halves[h2], ones_r) for h2 in range(2)]
            for h2 in range(2):
                _ln_chain(nc, l1sb, f"l1_{l}_{h2}", sums[h2][0], sums[h2][1],
                          halves[h2], eps_sb, ln1_out(h2))

        # ---- phase D: FFN + residual + LN2 -> x/xb ----
        for h2 in range(2):
            cs = slice(h2 * 512, (h2 + 1) * 512)
            with ExitStack() as dctx:
                dsb = dctx.enter_context(
                    tc.tile_pool(name=f"pd_sb{l}_{h2}", bufs=4))
                zps = dctx.enter_context(
                    tc.tile_pool(name=f"pd_psz{l}_{h2}", bufs=1, space="PSUM"))
                fps = dctx.enter_context(
                    tc.tile_pool(name=f"pd_psf{l}_{h2}", bufs=2, space="PSUM"))
                zp = [zps.tile([P, 512], F32, tag=f"z{mt}", name=f"z{mt}") for mt in range(FT)]
                for ms in range(FFT):
                    w1_sb = dsb.tile([P, HD], BF16, tag="w1s", name="w1s")
                    nc.sync.dma_start(w1_sb[:], t["w1"][l, ms])
                    fp = fps.tile([P, 512], F32, tag="fp", name="fp")
                    for kt in range(FT):
                        nc.tensor.matmul(fp[:], w1_sb[:, kt * P:(kt + 1) * P],
                                         yb[kt][:, cs], start=(kt == 0),
                                         stop=(kt == FT - 1))
                    f_sb = dsb.tile([P, 512], BF16, tag="fsb", name="fsb")
                    nc.scalar.activation(f_sb[:], fp[:], AF.Gelu,
                                         bias=b1_sb[:, ms:ms + 1])
                    w2_sb = dsb.tile([P, HD], BF16, tag="w2s", name="w2s")
                    nc.sync.dma_start(w2_sb[:],
                                      t["w2"][l, ms * P:(ms + 1) * P, :])
                    for mt in range(FT):
                        nc.tensor.matmul(zp[mt][:],
                                         w2_sb[:, mt * P:(mt + 1) * P],
                                         f_sb[:], start=(ms == 0),
                                         stop=(ms == FFT - 1))
                for mt in range(FT):
                    nc.vector.scalar_tensor_tensor(
                        out=r2[mt][:], in0=zp[mt][:],
                        scalar=b2_sb[:, mt:mt + 1], in1=yb[mt][:, cs],
                        op0=ALU.add, op1=ALU.add)

            def ln2_out(ft, src, h2=h2):
                off = h2 * 512
                nc.scalar.activation(xb[ft][:, W + off:W + off + 512], src,
                                     AF.Identity, scale=ls2_sb[:, ft:ft + 1],
                                     bias=lb2_sb[:, ft:ft + 1])
                nc.scalar.activation(x[ft][:, off:off + 512], src,
                                     AF.Identity, scale=ls2_sb[:, ft:ft + 1],
                                     bias=lb2_sb[:, ft:ft + 1])
            _ln_transposed(nc, tc, f"l2_{l}_{h2}",
                           [r2[ft][:] for ft in range(FT)],
                           ones_r, eps_sb, ln2_out)

            # Exchange left edges as soon as LN2 h0 lands: the AllGather of
            # left edges runs during FFN h1, and its scatter fills the RIGHT
            # halo (right neighbor's left edge) well before the next layer.
            if exchange and h2 == 0:
                b_in_l = edram.tile([FT, P, W], BF16, tag="binl", name="binl")
                b_out_l = edram.tile([4 * FT * P, W], BF16, tag="boutl",
                                     name="boutl")
                for ft in range(FT):
                    nc.sync.dma_start(b_in_l[ft], xb[ft][:, W:2 * W])
                nc.gpsimd.collective_compute(
                    "AllGather", ALU.bypass,
                    replica_groups=[[0, 1, 2, 3], [4, 5, 6, 7]],
                    ins=[b_in_l[:].opt()], outs=[b_out_l[:].opt()])
                for ft in range(FT):
                    nc.gpsimd.indirect_dma_start(
                        out=xb[ft][:, T_OWN + W:T_EXT], out_offset=None,
                        in_=b_out_l[:],
                        in_offset=bass.IndirectOffsetOnAxis(
                            ap=hid_sb[:, FT + ft:FT + ft + 1], axis=0))

        # ---- phase E: right-edge exchange (fills next layer's left halo) ----
        if exchange:
            b_in_r = edram.tile([FT, P, W], BF16, tag="binr", name="binr")
            b_out_r = edram.tile([4 * FT * P, W], BF16, tag="boutr",
                                 name="boutr")
            for ft in range(FT):
                nc.sync.dma_start(b_in_r[ft], xb[ft][:, T_OWN:T_OWN + W])
            nc.gpsimd.collective_compute(
                "AllGather", ALU.bypass,
                replica_groups=[[0, 1, 2, 3], [4, 5, 6, 7]],
                ins=[b_in_r[:].opt()], outs=[b_out_r[:].opt()])
            for ft in range(FT):
                nc.gpsimd.indirect_dma_start(
                    out=xb[ft][:, 0:W], out_offset=None, in_=b_out_r[:],
                    in_offset=bass.IndirectOffsetOnAxis(
                        ap=hid_sb[:, ft:ft + 1], axis=0))


# ---------------- host side ----------------

def _blocked(w, n_k, n_m):
    """[n_k*128, n_m*128] -> [n_m, 128, n_k, 128] (lhsT strips by out-tile)."""
    return np.ascontiguousarray(
        w.reshape(n_k, P, n_m, P).transpose(2, 1, 0, 3))


def _bias_lay(b, n):
    return np.ascontiguousarray(b.reshape(n, P).T)


def prepare(inputs):
    """Build per-core in_maps from full inputs."""
    ids_full = np.asarray(inputs["input_ids"]).astype(np.int32)
    am = np.asarray(inputs["attention_mask"]).astype(np.int32)
    emb_word = np.asarray(inputs["emb_word"], dtype=np.float32)
    emb_pos = np.asarray(inputs["emb_pos"], dtype=np.float32)
    Wq = np.asarray(inputs["Wq"], np.float32) / np.sqrt(DH)
    bq = np.asarray(inputs["bq"], np.float32) / np.sqrt(DH)
    Wk = np.asarray(inputs["Wk"], np.float32)
    bk = np.asarray(inputs["bk"], np.float32)
    Wv = np.asarray(inputs["Wv"], np.float32)
    bv = np.asarray(inputs["bv"], np.float32)
    Wo = np.asarray(inputs["Wo"], np.float32)
    bo = np.asarray(inputs["bo"], np.float32)
    W1 = np.asarray(inputs["W1"], np.float32)
    b1 = np.asarray(inputs["b1"], np.float32)
    W2 = np.asarray(inputs["W2"], np.float32)
    b2 = np.asarray(inputs["b2"], np.float32)
    assert np.all(am == 1), "general attention_mask needs mid-tile masks too"
    bf = ml_dtypes.bfloat16

    shared = {
        "emb_word": emb_word,
        "eln_s": np.asarray(inputs["emb_ln_s"], np.float32),
        "eln_b": np.asarray(inputs["emb_ln_b"], np.float32),
        "wq": np.stack([_blocked(Wq[i], FT, FT) for i in range(L)]).astype(bf),
        "wk": np.stack([_blocked(Wk[i], FT, FT) for i in range(L)]).astype(bf),
        "wv": Wv.astype(bf),
        "wo": np.stack([_blocked(Wo[i], FT, FT) for i in range(L)]).astype(bf),
        "w1": np.stack([_blocked(W1[i], FT, FFT) for i in range(L)]).astype(bf),
        "w2": W2.astype(bf),
        "bq": np.stack([_bias_lay(bq[i], FT) for i in range(L)]),
        "bk": np.stack([_bias_lay(bk[i], FT) for i in range(L)]),
        "bo": np.stack([_bias_lay(bv[i] @ Wo[i] + bo[i], FT)
                        for i in range(L)]),
        "b1": np.stack([_bias_lay(b1[i], FFT) for i in range(L)]),
        "b2": np.stack([_bias_lay(b2[i], FT) for i in range(L)]),
        "ls1": np.stack([_bias_lay(np.asarray(inputs["ln1_s"], np.float32)[i],
                                   FT) for i in range(L)]),
        "lb1": np.stack([_bias_lay(np.asarray(inputs["ln1_b"], np.float32)[i],
                                   FT) for i in range(L)]),
        "ls2": np.stack([_bias_lay(np.asarray(inputs["ln2_s"], np.float32)[i],
                                   FT) for i in range(L)]),
        "lb2": np.stack([_bias_lay(np.asarray(inputs["ln2_b"], np.float32)[i],
                                   FT) for i in range(L)]),
    }
    sel = np.zeros((NH, FT * P), np.float32)
    for ft in range(FT):
        sel[2 * ft, ft * P:ft * P + 64] = 1.0
        sel[2 * ft + 1, ft * P + 64:(ft + 1) * P] = 1.0
    shared["sel"] = sel.astype(bf)

    in_maps = []
    i_idx = np.arange(W)
    for core in range(N_CORES):
        b, sb = core // 4, core % 4
        s0 = sb * T_OWN
        ext_pos = np.clip(np.arange(s0 - W, s0 + T_OWN + W), 0, S - 1)
        m = dict(shared)
        m["ids"] = np.ascontiguousarray(
            ids_full[b, ext_pos].reshape(12, P).T)
        m["pos"] = np.ascontiguousarray(emb_pos[ext_pos])
        # masks: global chunk gc, window key j in [0,768), query i in [0,256):
        #   key_abs = gc*W - W + j ; allowed = |j - W - i| <= W
        #             & 0 <= key_abs < S & attention_mask[b, key_abs]
        mlm = np.zeros((NCH, P, 512), np.float32)
        mrm = np.zeros((NCH, P, 512), np.float32)
        for c in range(NCH):
            gc = sb * NCH + c
            for kt2 in range(2):
                for mm_, j0 in ((mlm, 0), (mrm, 512)):
                    j = j0 + kt2 * P + np.arange(P)[:, None]
                    key_abs = gc * W - W + j
                    ok = (np.abs(j - W - i_idx[None, :]) <= W)
                    ok &= (key_abs >= 0) & (key_abs < S)
                    ok &= am[b, np.clip(key_abs, 0, S - 1)] > 0
                    mm_[c, :, kt2 * W:(kt2 + 1) * W] = ok
        m["ml"] = mlm.astype(bf)
        m["mr"] = mrm.astype(bf)
        # halo row ids: cols 0-5 index b_out_r (right edges; left halo comes
        # from left neighbor's right edge), cols 6-11 index b_out_l (left
        # edges; right halo from right neighbor's left edge). Both tables are
        # [4, FT, 128, W] row-major. Edge cores fall back to their own rows
        # (bounded values; masked out of the attention anyway).
        hid = np.zeros((2, FT, P), np.int64)
        for side in range(2):
            nb = sb - 1 if side == 0 else sb + 1
            src = nb if 0 <= nb <= 3 else sb
            for ft in range(FT):
                hid[side, ft] = (src * FT + ft) * P + np.arange(P)
        m["halo_ids"] = np.ascontiguousarray(
            hid.reshape(12, P).T.astype(np.int32))
        in_maps.append(m)
    return in_maps


_NC_CACHE = {}


def get_nc(n_layers=L):
    if n_layers not in _NC_CACHE:
        _NC_CACHE[n_layers] = build_nc(n_layers)
    return _NC_CACHE[n_layers]


def run(inputs, n_layers=L, trace=False):
    nc = get_nc(n_layers)
    in_maps = prepare(inputs)
    res = bass_utils.run_bass_kernel_spmd(
        nc, in_maps, core_ids=list(range(N_CORES)), trace=trace)
    outs = np.empty((B, S, HD), np.float32)
    for core in range(N_CORES):
        b, sb = core // 4, core % 4
        ot = res.results[core]["out"]  # [FT, 128, T_OWN]
        outs[b, sb * T_OWN:(sb + 1) * T_OWN] = ot.reshape(HD, T_OWN).T
    return outs, res


def kernel(**inputs) -> np.ndarray:
    out, _ = run(inputs)
    return out
